# revision 1
# baseline (speedup 1.0000x reference)
"""GNN message-passing discriminator on 8 trn2 NeuronCores.

Strategy (edge-parallel by *destination* node):
  - Nodes sharded npc=6250/core; each edge lives on the core owning its dst.
  - Each core keeps a replicated node-feature table in SBUF (fp16, 128-col
    256B slots, int16-addressable in two buckets) and gathers x_j
    feature-major with transposed SBUF-source dma_gather.
  - Message MLP: TensorE matmul per 128-edge tile (gathered tile is the
    stationary operand; edge_attr rows are appended below the features).
    LeakyReLU and the mean's 1/deg(dst) scale fuse into one ScalarE
    activation per tile.
  - Aggregation: per-tile one-hot fp8 selector matrices (host-built,
    streamed from HBM) matmul'd against messages, accumulating
    feature-major per-128-node-window sums in PSUM.
  - Update MLP per window; h_next is AllGather'd between layers to rebuild
    the table.
  - Global mean-pool via per-window batch-selector matmuls; final MLP runs
    replicated on every core.

Host-side work is integer index prep (sort/bincount/one-hot selectors) and
layout/dtype staging; all float compute runs on device.
"""

import numpy as np

DEBUG = False

import concourse.bass as bass
import concourse.bacc as bacc
import concourse.mybir as mybir
import concourse.tile as tile
from concourse.bass_utils import run_bass_kernel_spmd

F32 = mybir.dt.float32
F16 = mybir.dt.float16
F8 = mybir.dt.float8e4
I16 = mybir.dt.int16
AF = mybir.ActivationFunctionType
NP_F8 = mybir.dt.np(F8)

N_GRAPHS = 32
HID = [64, 128, 256]
MLP_DIMS = [256, 128, 64, 1]
N_CORES = 8

ELEM = 128      # fp16 feature slots per table entry (256 bytes)
WIN = 128       # nodes per aggregation window
GROUP_W = 4     # windows per gather group
LRELU = 0.2


def _cdiv(a, b):
    return -(-a // b)


_LRELU_OP = None


def _get_lrelu_op():
    """out = max(s*x, 0.2*s*x) in one DVE pass (s per-partition, 0.2 imm)."""
    global _LRELU_OP
    if _LRELU_OP is not None:
        return _LRELU_OP
    import concourse.dve_ops as dops
    from concourse.dve_spec import Spec, Src0, C0, C2, maxx
    name = "LRELU_SCALE_ANT"
    if name not in dops._SUB_OPCODE_FOR_NAME:
        row = max(dops._SUB_OPCODE_FOR_NAME.values()) + 1
        assert row < 0x20
        dops._SUB_OPCODE_FOR_NAME[name] = row
    spec = Spec(
        body=maxx(Src0 * C0, Src0 * C0 * C2),
        reference=lambda in0, in1, c0, c1, c2: np.maximum(
            in0 * c0, in0 * c0 * c2),
    )
    shas = {}
    for ver in ("v3", "v4"):
        try:
            probe = dops.DveOp(name, spec, subdim=False, uops_sha={})
            probe.compile(ver)
        except ValueError as ex:
            import re
            m = re.search(r"\{}: ([0-9a-f]{{16}})".format(ver), str(ex))
            if not m:
                m = re.search(r"([0-9a-f]{16}) \u2260|([0-9a-f]{16}) ", str(ex))
            shas[ver] = re.search(r"\(" + ver + r": ([0-9a-f]+)", str(ex)).group(1)
    op = dops.DveOp(name, spec, subdim=False, uops_sha=shas)
    if not any(o.name == name for o in dops.OPS):
        dops.OPS.append(op)
    dops.CUSTOM_DVE_SPECS[name] = spec
    _LRELU_OP = op
    return op


class Cfg:
    pass


# ============================================================ host index prep
def host_prep(inputs, n_cores=N_CORES):
    x = np.asarray(inputs["x"], np.float32)
    ei = np.asarray(inputs["edge_index"], np.int64)
    ea = np.asarray(inputs["edge_attr"], np.float32)
    batch = np.asarray(inputs["batch"], np.int64)

    n_nodes, node_dim = x.shape
    n_edges = ei.shape[1]

    cfg = Cfg()
    cfg.n_cores = n_cores
    cfg.n_nodes = n_nodes
    cfg.node_dim = node_dim
    cfg.n_graphs = N_GRAPHS
    npc = n_nodes // n_cores
    assert npc * n_cores == n_nodes
    cfg.npc = npc
    lo = min(_cdiv(_cdiv(npc, 2), 128) * 128, npc)
    hi = _cdiv(npc - lo, 128) * 128
    cfg.lo, cfg.hi = lo, hi
    cfg.hi_used = npc - lo
    cfg.lo_total = lo * n_cores
    cfg.hi_total = hi * n_cores
    cfg.slots = cfg.lo_total + cfg.hi_total
    cfg.stripes = cfg.slots // 128
    cfg.lo_stripes = cfg.lo_total // 128
    assert cfg.lo_total < 32768 and cfg.hi_total < 32768
    cfg.n_win = _cdiv(npc, WIN)
    cfg.last_win_nodes = npc - (cfg.n_win - 1) * WIN

    src = ei[0].astype(np.int64)
    dst = ei[1].astype(np.int64)
    deg = np.bincount(dst, minlength=n_nodes).astype(np.float32)
    inv_deg = (1.0 / np.maximum(deg, 1.0)).astype(np.float32)

    c_of = np.arange(n_nodes) // npc
    r_of = np.arange(n_nodes) % npc
    slot = np.where(
        r_of < lo,
        c_of * lo + r_of,
        cfg.lo_total + c_of * hi + (r_of - lo),
    ).astype(np.int64)
    slot_of_node = slot

    sslot = slot[src]
    ecore = dst // npc
    ewin = (dst % npc) // WIN
    ebuck = (sslot >= cfg.lo_total).astype(np.int64)

    key = (ecore * cfg.n_win + ewin) * 2 + ebuck
    cnt = np.bincount(key, minlength=n_cores * cfg.n_win * 2).reshape(
        n_cores, cfg.n_win, 2)
    T = np.maximum(_cdiv(cnt.max(axis=0), 128), 1)   # [n_win, 2]
    cfg.T = T
    cfg.n_tiles = int(T.sum())
    cfg.e_pad = cfg.n_tiles * 128

    groups = [list(range(g, min(g + GROUP_W, cfg.n_win)))
              for g in range(0, cfg.n_win, GROUP_W)]
    cfg.groups = groups

    # padded stream order: per group: [A segs of its windows] [B segs]
    seg_off = {}
    pos = 0
    for ws in groups:
        for b in (0, 1):
            for w in ws:
                seg_off[(w, b)] = pos
                pos += int(T[w, b]) * 128
    assert pos == cfg.e_pad
    cfg.seg_off = seg_off

    order = np.lexsort((ebuck, ewin, ecore))
    src_s = sslot[order]
    dst_s = dst[order]
    ea_s = ea[order]
    inv_s = inv_deg[dst[order]]

    ck = (ecore[order] * cfg.n_win + ewin[order]) * 2 + ebuck[order]
    seg_starts = np.searchsorted(ck, np.arange(n_cores * cfg.n_win * 2))
    seg_ends = np.append(seg_starts[1:], n_edges)

    win_pad = cfg.n_win * WIN
    e_pad = cfg.e_pad
    in_maps = []
    wts = _pack_weights(inputs, node_dim)
    ident = np.eye(128, dtype=np.float16)

    for c in range(n_cores):
        g_idx = np.zeros(e_pad, np.int64)
        buck_flag = np.zeros(e_pad, np.bool_)
        e_a = np.zeros((4, e_pad), np.float32)
        invd = np.zeros(e_pad, np.float32)
        selcol = np.full(e_pad, -1, np.int64)

        for w in range(cfg.n_win):
            for b in (0, 1):
                s0 = seg_starts[(c * cfg.n_win + w) * 2 + b]
                s1 = seg_ends[(c * cfg.n_win + w) * 2 + b]
                n = s1 - s0
                o = seg_off[(w, b)]
                assert n <= T[w, b] * 128
                if n:
                    buck_flag[o:o + n] = bool(b)
                    g_idx[o:o + n] = src_s[s0:s1] - (cfg.lo_total if b else 0)
                    e_a[:3, o:o + n] = ea_s[s0:s1].T
                    e_a[3, o:o + n] = 1.0
                    invd[o:o + n] = inv_s[s0:s1]
                    selcol[o:o + n] = (dst_s[s0:s1] % npc) - w * WIN

        gi = np.zeros((128, e_pad // 16), np.int16)
        base = g_idx.astype(np.int16).reshape(-1, 16).T
        for k in range(8):
            gi[16 * k:16 * k + 16] = base

        # layer-0 edge stream: [x[src](10) | ea(3) | 1] fp16, feature-major
        xe = np.zeros((node_dim + 4, e_pad), np.float16)
        edge_valid = selcol >= 0
        # recover per-edge src x via the slot->x map
        xe[:node_dim, :] = 0.0

        sel = np.zeros((128, cfg.n_tiles * 128), np.uint8)
        tt = np.arange(e_pad) // 128
        ee = np.arange(e_pad) % 128
        m = selcol >= 0
        sel[ee[m], tt[m] * 128 + selcol[m]] = 0x38

        xt = np.zeros((node_dim, win_pad), np.float16)
        xt[:, :npc] = x[c * npc:(c + 1) * npc].astype(np.float16).T

        sb = np.zeros((128, cfg.n_win * N_GRAPHS), np.uint8)
        bl = batch[c * npc:(c + 1) * npc].astype(np.int64)
        pp = np.arange(npc) % WIN
        ww = np.arange(npc) // WIN
        sb[pp, ww * N_GRAPHS + bl] = 0x38

        xsrc_slot = np.zeros((cfg.slots, node_dim), np.float16)
        xsrc_slot[slot_of_node] = x.astype(np.float16)
        gsl = g_idx + np.where(buck_flag, cfg.lo_total, 0)
        xe[:node_dim, :] = xsrc_slot[gsl].T
        xe[node_dim:node_dim + 4, :] = e_a.astype(np.float16)
        xe[:, ~edge_valid] = 0.0

        m_ = {
            "xeT": xe,
            "gidx": gi,
            "eaT": e_a.astype(np.float16),
            "invd": invd.reshape(-1, 128).T.astype(np.float32).copy(),
            "sel": sel.view(NP_F8),
            "xT_loc": xt,
            "selB": sb.view(NP_F8),
            "ident": ident,
        }
        m_.update(wts)
        in_maps.append(m_)
    return cfg, in_maps


def _pack_weights(inputs, node_dim):
    wts = {}
    node_in = node_dim
    for li in range(len(HID)):
        mw = np.asarray(inputs[f"mw{li}"], np.float32)
        mb = np.asarray(inputs[f"mb{li}"], np.float32)
        wts[f"mwp{li}"] = np.concatenate([mw, mb[None, :]], axis=0)
        wts[f"uw{li}"] = np.asarray(inputs[f"uw{li}"], np.float32)
        wts[f"ub{li}"] = np.asarray(inputs[f"ub{li}"], np.float32)[None, :]
        node_in = HID[li]
    for li in range(len(MLP_DIMS) - 1):
        wts[f"fw{li}"] = np.asarray(inputs[f"fw{li}"], np.float32)
        wts[f"fb{li}"] = np.asarray(
            inputs[f"fb{li}"], np.float32).reshape(-1, 1)
    return wts


# =============================================================== bass builder
def build_program(cfg):
    nc = bacc.Bacc(
        "TRN2",
        target_bir_lowering=False,
        debug=False,
        enable_asserts=False,
        num_devices=cfg.n_cores,
    )
    n_win, npc, n_tiles, e_pad = cfg.n_win, cfg.npc, cfg.n_tiles, cfg.e_pad
    slots, stripes, lo_stripes = cfg.slots, cfg.stripes, cfg.lo_stripes
    win_pad = n_win * WIN
    NG = cfg.n_graphs
    T = cfg.T
    groups = cfg.groups
    seg_off = cfg.seg_off
    core_ids = list(range(cfg.n_cores))

    D = {}

    def din(name, shape, dt):
        D[name] = nc.dram_tensor(name, list(shape), dt, kind="ExternalInput")

    din("xeT", (cfg.node_dim + 4, e_pad), F16)
    din("gidx", (128, e_pad // 16), I16)
    din("eaT", (4, e_pad), F16)
    din("invd", (128, n_tiles), F32)
    din("sel", (128, n_tiles * 128), F8)
    din("xT_loc", (cfg.node_dim, win_pad), F16)
    din("selB", (128, n_win * NG), F8)
    din("ident", (128, 128), F16)
    node_in = cfg.node_dim
    for li, dout in enumerate(HID):
        din(f"mwp{li}", (node_in + 4, dout), F32)
        din(f"uw{li}", (dout + node_in, dout), F32)
        din(f"ub{li}", (1, dout), F32)
        node_in = dout
    for li in range(len(MLP_DIMS) - 1):
        din(f"fw{li}", (MLP_DIMS[li], MLP_DIMS[li + 1]), F32)
        din(f"fb{li}", (MLP_DIMS[li + 1], 1), F32)
    out_t = nc.dram_tensor("out", [NG, 1], F32, kind="ExternalOutput")

    # group extents in the padded stream
    g_meta = []
    for ws in groups:
        nA = int(sum(T[w, 0] for w in ws)) * 128
        nB = int(sum(T[w, 1] for w in ws)) * 128
        g_meta.append((seg_off[(ws[0], 0)], nA, nB))
    max_g_cols = max(nA + nB for _, nA, nB in g_meta)
    max_w_cols = int((T[:, 0] + T[:, 1]).max()) * 128

    from contextlib import ExitStack
    with ExitStack() as _es:
        tc = _es.enter_context(tile.TileContext(nc))
        p_table = _es.enter_context(tc.tile_pool(name="table", bufs=1))
        p_res = _es.enter_context(tc.tile_pool(name="res", bufs=1))
        p_wts = _es.enter_context(tc.tile_pool(name="wts", bufs=1))
        p_gath = _es.enter_context(tc.tile_pool(name="gath", bufs=2))
        p_sel = _es.enter_context(tc.tile_pool(name="selp", bufs=2))
        p_gix = _es.enter_context(tc.tile_pool(name="gix", bufs=2))
        p_ivd = _es.enter_context(tc.tile_pool(name="ivd", bufs=2))
        p_ea = _es.enter_context(tc.tile_pool(name="eal2", bufs=2))
        p_msg = _es.enter_context(tc.tile_pool(name="msg", bufs=6))
        p_aggs = _es.enter_context(tc.tile_pool(name="aggs", bufs=4))
        p_hloc = _es.enter_context(tc.tile_pool(name="hloc", bufs=2))
        p_hn = _es.enter_context(tc.tile_pool(name="hnext", bufs=3))
        p_small = _es.enter_context(tc.tile_pool(name="small", bufs=1))
        pp_msg = _es.enter_context(tc.tile_pool(name="pmsg", bufs=3, space="PSUM"))
        pp_agg = _es.enter_context(tc.tile_pool(name="pagg", bufs=2, space="PSUM"))
        pp_upd = _es.enter_context(tc.tile_pool(name="pupd", bufs=2, space="PSUM"))
        pp_pool = _es.enter_context(tc.tile_pool(name="ppool", bufs=1, space="PSUM"))
        p_dram = _es.enter_context(tc.tile_pool(name="dram", bufs=1, space="DRAM"))
        if True:
            lo_s = lo_stripes
            hi_s = stripes - lo_stripes
            tableA = p_table.tile([128, lo_s * ELEM], F16, tag="tabA")
            tableB = p_table.tile([128, hi_s * ELEM], F16, tag="tabB")
            tblA3 = tableA[:].rearrange("p (s c) -> p s c", c=ELEM)
            tblB3 = tableB[:].rearrange("p (s c) -> p s c", c=ELEM)
            selB_sb = p_res.tile([128, n_win * NG], F8, tag="selB")
            ident_sb = p_res.tile([128, 128], F16, tag="ident")
            ones_row = p_res.tile([1, 128], F16, tag="ones_r")
            ones_col = p_res.tile([128, 1], F16, tag="ones_c")

            nc.sync.dma_start(selB_sb[:], D["selB"][:])
            nc.sync.dma_start(ident_sb[:], D["ident"][:])
            nc.vector.memset(ones_row[:], 1.0)
            nc.vector.memset(ones_col[:], 1.0)

            # weights -> SBUF fp16 (cast during SWDGE DMA)
            W = {}
            node_in = cfg.node_dim
            for li, dout in enumerate(HID):
                mw_chunks = []
                for k, r in enumerate(range(0, node_in + 4, 128)):
                    r1 = min(r + 128, node_in + 4)
                    t = p_wts.tile([r1 - r, dout], F16, tag=f"mwp{li}_{k}")
                    nc.gpsimd.dma_start(t[:], D[f"mwp{li}"][r:r1, :])
                    mw_chunks.append(t)
                W[f"mwp{li}"] = mw_chunks
                # uw chunks: agg rows [0:dout] in 128-chunks, then h rows
                chunks = []
                for r in list(range(0, dout, 128)):
                    chunks.append((r, min(r + 128, dout)))
                for r in list(range(0, node_in, 128)):
                    chunks.append((dout + r, dout + min(r + 128, node_in)))
                uws = []
                for k, (r0, r1) in enumerate(chunks):
                    t = p_wts.tile([r1 - r0, dout], F16, tag=f"uw{li}_{k}")
                    nc.gpsimd.dma_start(t[:], D[f"uw{li}"][r0:r1, :])
                    uws.append(t)
                W[f"uw{li}"] = uws
                t = p_wts.tile([1, dout], F16, tag=f"ub{li}")
                nc.gpsimd.dma_start(t[:], D[f"ub{li}"][:])
                W[f"ub{li}"] = t
                node_in = dout
            for li in range(len(MLP_DIMS) - 1):
                fws = []
                for k, r in enumerate(range(0, MLP_DIMS[li], 128)):
                    r1 = min(r + 128, MLP_DIMS[li])
                    t = p_wts.tile([r1 - r, MLP_DIMS[li + 1]], F16,
                                   tag=f"fw{li}_{k}")
                    nc.gpsimd.dma_start(t[:], D[f"fw{li}"][r:r1, :])
                    fws.append(t)
                W[f"fw{li}"] = fws
                t = p_wts.tile([MLP_DIMS[li + 1], 1], F32, tag=f"fb{li}")
                nc.sync.dma_start(t[:], D[f"fb{li}"][:])
                W[f"fb{li}"] = t

            hlocT = p_hloc.tile([cfg.node_dim, win_pad], F16, tag="hloc")
            nc.sync.dma_start(hlocT[:], D["xT_loc"][:])
            nc.gpsimd.memset(tableA[:], 0.0)
            nc.gpsimd.memset(tableB[:], 0.0)
            zeros_sb = p_res.tile([128, max(HID)], F16, tag="zeros")
            nc.vector.memset(zeros_sb[:], 0.0)

            lo_w = cfg.lo // 128
            assert cfg.hi_total > 0
            hi_nw = n_win - lo_w
            amid = max(1, lo_w // 2)
            hmid = lo_w + max(1, hi_nw // 2)
            chunk_bounds = [(0, amid), (amid, lo_w),
                            (lo_w, min(hmid, n_win)),
                            (min(hmid, n_win), n_win)]
            chunk_bounds = [(a, b) for a, b in chunk_bounds if b > a]
            chunk_of_w = {}
            for ci, (a, b) in enumerate(chunk_bounds):
                for w in range(a, b):
                    chunk_of_w[w] = ci
            ag_in = {}
            ag_out = {}
            for li in (0, 1):
                for ci, (a, b) in enumerate(chunk_bounds):
                    ag_in[(li, ci)] = p_dram.tile(
                        [128, (b - a) * HID[li]], F16,
                        tag=f"agi{li}_{ci}", name=f"agi{li}_{ci}")
                    ag_out[(li, ci)] = p_dram.tile(
                        [cfg.n_cores * 128, (b - a) * HID[li]], F16,
                        tag=f"ago{li}_{ci}", name=f"ago{li}_{ci}",
                        addr_space="Shared")
            FP = HID[2] + 1
            gp_in = p_dram.tile([NG, FP], F32, tag="gpi")
            if DEBUG:
                dbg_h3 = p_dram.tile([128, n_win * HID[2]], F16, tag="dbgh3",
                                     name="dbg_h3")
                dbg_ag = p_dram.tile([128, n_win * HID[2]], F16, tag="dbgag",
                                     name="dbg_ag")
                dbg_msg = p_dram.tile([128, 16 * HID[2]], F16, tag="dbgmsg",
                                      name="dbg_msg")
            gp_out = p_dram.tile([cfg.n_cores * NG, FP], F32, tag="gpo", addr_space="Shared")

            psum_pool = pp_pool.tile([NG, FP], F32)

            lo_s_pc = cfg.lo // 128
            hi_s_pc = n_win - lo_s_pc

            def _ag_emit(li, ci, dout):
                nc.gpsimd.collective_compute(
                    "AllGather",
                    mybir.AluOpType.bypass,
                    replica_groups=[core_ids],
                    ins=[ag_in[(li, ci)].opt()],
                    outs=[ag_out[(li, ci)].opt()],
                )

            def _rebuild_emit(li, dout):
                # deferred: overwrites the tables, so must come after the
                # layer's last gather (Tile WAR-orders it automatically)
                for ci, (a, b) in enumerate(chunk_bounds):
                    nw = b - a
                    for rr in range(cfg.n_cores):
                        frag = ag_out[(li, ci)][rr * 128:(rr + 1) * 128, :]\
                            .rearrange("p (s c) -> p s c", c=dout)
                        if b <= lo_w:
                            s0 = rr * lo_s_pc + a
                            nc.sync.dma_start(
                                tblA3[:, s0:s0 + nw, 0:dout], frag[:])
                        else:
                            s0 = rr * hi_s_pc + (a - lo_w)
                            nc.sync.dma_start(
                                tblB3[:, s0:s0 + nw, 0:dout], frag[:])

            # ========================= layers
            node_in = cfg.node_dim
            for li, dout in enumerate(HID):
                mwp = W[f"mwp{li}"]
                nch = _cdiv(dout, 128)
                last = li == len(HID) - 1
                hnextT = None
                if not last:
                    hnextT = p_hloc.tile([dout, win_pad], F16, tag="hloc")

                for gi_, ws in enumerate(groups):
                    start, nA, nB = g_meta[gi_]
                    ncols = nA + nB
                    selbuf = p_sel.tile([128, max_g_cols], F8, tag="selp")
                    ivdb = p_ivd.tile([128, max_g_cols // 128], F32, tag="ivd")
                    nc.sync.dma_start(
                        ivdb[:, 0:ncols // 128],
                        D["invd"][:, start // 128:(start + ncols) // 128])
                    nc.sync.dma_start(
                        selbuf[:, 0:ncols],
                        D["sel"][:, start:start + ncols])
                    if li == 0:
                        gbuf = p_gath.tile([cfg.node_dim + 4, max_g_cols],
                                           F16, tag="gath", name="gbuf")
                        nc.sync.dma_start(
                            gbuf[:, 0:ncols],
                            D["xeT"][:, start:start + ncols])
                    else:
                        gbuf = p_gath.tile([128, max_g_cols], F16, tag="gath",
                                           name="gbuf")
                        gixb = p_gix.tile([128, max_g_cols // 16], I16,
                                          tag="gix")
                        nc.sync.dma_start(
                            gixb[:, 0:ncols // 16],
                            D["gidx"][:, start // 16:(start + ncols) // 16])
                        for b, coff, n_b in ((0, 0, nA), (1, nA, nB)):
                            if n_b == 0:
                                continue
                            tb = tableA[:] if b == 0 else tableB[:]
                            nc.gpsimd.dma_gather(
                                gbuf[:, coff:coff + n_b].rearrange(
                                    "p (o n) -> p o n", o=1),
                                tb,
                                gixb[:, coff // 16:(coff + n_b) // 16],
                                num_idxs=n_b,
                                num_idxs_reg=n_b,
                                elem_size=ELEM,
                                transpose=True,
                                single_packet=False,
                                sbuf_tokens_per_rank=128,
                                sbuf_free_dim_per_rank=ELEM * 2,
                            )
                        if node_in + 4 <= 128:
                            nc.sync.dma_start(
                                gbuf[node_in:node_in + 4, 0:ncols],
                                D["eaT"][:, start:start + ncols])

                    # window-major processing; stream offsets are bucket-major
                    aoff = 0
                    boff = nA
                    for w in ws:
                        t0, t1 = int(T[w, 0]), int(T[w, 1])
                        wcols = []
                        for k in range(t0):
                            wcols.append(aoff + k * 128)
                        for k in range(t1):
                            wcols.append(boff + k * 128)
                        aoff += t0 * 128
                        boff += t1 * 128

                        eabuf = None
                        if node_in + 4 > 128:
                            eabuf = p_ea.tile([4, max_w_cols], F16, tag="eal2")
                            nc.sync.dma_start(
                                eabuf[:, 0:t0 * 128],
                                D["eaT"][:, start + wcols[0]:
                                         start + wcols[0] + t0 * 128])
                            nc.sync.dma_start(
                                eabuf[:, t0 * 128:(t0 + t1) * 128],
                                D["eaT"][:, start + wcols[t0]:
                                         start + wcols[t0] + t1 * 128])

                        pagg = pp_agg.tile([128, nch * 128], F32, tag="pagg")
                        n_wt = len(wcols)
                        pend = None

                        def _sel_emit(pk, pcol, pmsg_t):
                            for ch in range(nch):
                                c0, c1 = ch * 128, min(dout, ch * 128 + 128)
                                nc.tensor.matmul(
                                    pagg[0:c1 - c0, ch * 128:ch * 128 + 128],
                                    pmsg_t[:, c0:c1],
                                    selbuf[:, pcol:pcol + 128],
                                    start=(pk == 0 and ch == 0),
                                    stop=(pk == n_wt - 1 and ch == nch - 1),
                                    skip_group_check=True)

                        for k, col in enumerate(wcols):
                            pmsg = pp_msg.tile([128, dout], F32, tag="pmsg")
                            if node_in + 4 <= 128:
                                nc.tensor.matmul(
                                    pmsg[:], gbuf[0:node_in + 4, col:col + 128],
                                    mwp[0][:], start=True, stop=True)
                            else:
                                nc.tensor.matmul(
                                    pmsg[:], gbuf[0:128, col:col + 128],
                                    mwp[0][:], start=True, stop=False)
                                nc.tensor.matmul(
                                    pmsg[:], eabuf[:, k * 128:k * 128 + 128],
                                    mwp[1][:], start=False, stop=True)
                            msg = p_msg.tile([128, dout], F16, tag="msg")
                            tloc = col // 128
                            sc_ap = ivdb[:, tloc:tloc + 1]
                            if k % 2 == 0:
                                nc.scalar.activation(
                                    msg[:], pmsg[:], AF.Prelu,
                                    scale=sc_ap, alpha=LRELU)
                            else:
                                nc.vector._custom_dve(
                                    _get_lrelu_op(), out=msg[:],
                                    in0=pmsg[:], s0=sc_ap, imm2=LRELU)
                            if DEBUG and last and w == 0 and k < 16:
                                nc.sync.dma_start(
                                    dbg_msg[:, k * dout:(k + 1) * dout],
                                    msg[:])
                            if pend is not None:
                                _sel_emit(*pend)
                            pend = (k, col, msg)
                        if pend is not None:
                            _sel_emit(*pend)

                        # ----- window update
                        wn = WIN if w < n_win - 1 else cfg.last_win_nodes
                        lhs_list = []
                        for ch in range(nch):
                            c0, c1 = ch * 128, min(dout, ch * 128 + 128)
                            a = p_aggs.tile([128, 128], F16, tag="aggs")
                            nc.vector.tensor_copy(
                                a[0:c1 - c0, :],
                                pagg[0:c1 - c0, ch * 128:ch * 128 + 128])
                            lhs_list.append(a[0:c1 - c0, :])
                        for r in range(0, node_in, 128):
                            r1 = min(node_in, r + 128)
                            lhs_list.append(
                                hlocT[r:r1, w * WIN:w * WIN + 128])
                        uws = W[f"uw{li}"]
                        assert len(uws) == len(lhs_list)
                        pupd = pp_upd.tile([128, max(HID)], F32, tag="pupd")
                        for i, lhs in enumerate(lhs_list):
                            nc.tensor.matmul(
                                pupd[:, 0:dout], lhs, uws[i][:],
                                start=(i == 0), stop=False)
                        nc.tensor.matmul(
                            pupd[:, 0:dout], ones_row[:], W[f"ub{li}"][:],
                            start=False, stop=True)
                        hn = p_hn.tile([128, max(HID)], F16, tag="hnext")
                        nc.scalar.activation(
                            hn[:, 0:dout], pupd[:, 0:dout], AF.Prelu,
                            alpha=LRELU)
                        if not last:
                            ci = chunk_of_w[w]
                            wl = w - chunk_bounds[ci][0]
                            agi = ag_in[(li, ci)]
                            nc.sync.dma_start(
                                agi[:, wl * dout:wl * dout + dout]
                                if wn == WIN else
                                agi[0:wn, wl * dout:wl * dout + dout],
                                hn[:, 0:dout] if wn == WIN
                                else hn[0:wn, 0:dout])
                            if wn < WIN:
                                nc.sync.dma_start(
                                    agi[wn:128,
                                        wl * dout:wl * dout + dout],
                                    zeros_sb[wn:128, 0:dout])
                            for ch in range(nch):
                                c0, c1 = ch * 128, min(dout, ch * 128 + 128)
                                pt = pp_upd.tile([128, 128], F16, tag="pupd")
                                nc.tensor.transpose(
                                    pt[0:c1 - c0, :], hn[:, c0:c1],
                                    ident_sb[:])
                                nc.vector.tensor_copy(
                                    hnextT[c0:c1, w * WIN:w * WIN + 128],
                                    pt[0:c1 - c0, :])
                            if w == chunk_bounds[ci][1] - 1:
                                _ag_emit(li, ci, dout)
                        else:
                            if DEBUG:
                                nc.sync.dma_start(
                                    dbg_h3[:, w * dout:(w + 1) * dout],
                                    hn[:, 0:dout])
                                for _ch in range(nch):
                                    _cw = (min(dout, _ch * 128 + 128)
                                           - _ch * 128)
                                    nc.sync.dma_start(
                                        dbg_ag[0:_cw,
                                               (w * 2 + _ch) * 128:
                                               (w * 2 + _ch) * 128 + 128],
                                        lhs_list[_ch])
                            nc.tensor.matmul(
                                psum_pool[:, 0:dout],
                                selB_sb[:, w * NG:(w + 1) * NG],
                                hn[:, 0:dout],
                                start=(w == 0), stop=False,
                                skip_group_check=True)
                            nc.tensor.matmul(
                                psum_pool[:, dout:dout + 1],
                                selB_sb[:, w * NG:(w + 1) * NG],
                                ones_col[:],
                                start=False, stop=(w == n_win - 1),
                                skip_group_check=True)

                # ----- end of layer
                if not last:
                    _rebuild_emit(li, dout)
                    hlocT = hnextT
                node_in = dout

            # ========================= pooling finale + MLP (replicated)
            gp_sb = p_gath.tile([NG, FP], F32, tag="gath", name="gp_sb")
            nc.vector.tensor_copy(gp_sb[:], psum_pool[:])
            nc.sync.dma_start(gp_in[:], gp_sb[:])
            nc.gpsimd.collective_compute(
                "AllGather",
                mybir.AluOpType.bypass,
                replica_groups=[core_ids],
                ins=[gp_in.opt()],
                outs=[gp_out.opt()],
            )
            gall = p_gath.tile([NG, FP * cfg.n_cores], F32, tag="gath", name="gall")
            nc.sync.dma_start(
                gall[:].rearrange("g (f r) -> g f r", r=cfg.n_cores),
                gp_out[:].rearrange("(r g) f -> g f r", g=NG))
            gsum = p_gath.tile([NG, FP], F32, tag="gath", name="gsum")
            nc.vector.tensor_reduce(
                gsum[:],
                gall[:].rearrange("g (f r) -> g f r", r=cfg.n_cores),
                axis=mybir.AxisListType.X,
                op=mybir.AluOpType.add)
            rec = p_small.tile([NG, 1], F32, tag="rec")
            nc.vector.reciprocal(rec[:], gsum[:, HID[2]:HID[2] + 1])
            g16 = p_small.tile([NG, HID[2]], F16, tag="g16")
            nc.scalar.activation(g16[:], gsum[:, 0:HID[2]], AF.Identity,
                                 scale=rec[:])
            gT = p_small.tile([128, 2 * NG], F16, tag="gT")
            for ch in range(2):
                pt = pp_upd.tile([128, 128], F16, tag="pupd")
                nc.tensor.transpose(
                    pt[0:128, 0:NG], g16[:, ch * 128:(ch + 1) * 128],
                    ident_sb[0:NG, 0:NG])
                nc.vector.tensor_copy(
                    gT[:, ch * NG:(ch + 1) * NG], pt[0:128, 0:NG])
            pf1 = pp_upd.tile([128, NG], F32, tag="pupd")
            for ch in range(2):
                nc.tensor.matmul(
                    pf1[0:MLP_DIMS[1], :], W["fw0"][ch][:],
                    gT[:, ch * NG:(ch + 1) * NG],
                    start=(ch == 0), stop=(ch == 1))
            f1 = p_small.tile([MLP_DIMS[1], NG], F16, tag="f1")
            nc.scalar.activation(f1[:], pf1[0:MLP_DIMS[1], :], AF.Prelu,
                                 bias=W["fb0"][:], alpha=LRELU)
            pf2 = pp_upd.tile([128, NG], F32, tag="pupd")
            nc.tensor.matmul(pf2[0:MLP_DIMS[2], :], W["fw1"][0][:], f1[:],
                             start=True, stop=True)
            f2 = p_small.tile([MLP_DIMS[2], NG], F16, tag="f2")
            nc.scalar.activation(f2[:], pf2[0:MLP_DIMS[2], :], AF.Prelu,
                                 bias=W["fb1"][:], alpha=LRELU)
            pf3 = pp_upd.tile([128, NG], F32, tag="pupd")
            nc.tensor.matmul(pf3[0:1, :], W["fw2"][0][:], f2[:],
                             start=True, stop=True)
            fout = p_small.tile([1, NG], F32, tag="fo")
            nc.scalar.activation(fout[:], pf3[0:1, :], AF.Identity,
                                 bias=W["fb2"][:])
            nc.sync.dma_start(out_t[:].rearrange("g o -> o g"), fout[:])

    return nc


def kernel(**inputs):
    cfg, in_maps = host_prep(inputs)
    nc = build_program(cfg)
    nc.compile()
    res = run_bass_kernel_spmd(nc, in_maps, core_ids=list(range(cfg.n_cores)))
    return np.asarray(res.results[0]["out"], np.float32)



# revision 11
# speedup vs baseline: 1.0306x; 1.0306x over previous
"""GNN message-passing discriminator on 8 trn2 NeuronCores.

Strategy (edge-parallel by *destination* node):
  - Nodes sharded npc=6250/core; each edge lives on the core owning its dst.
  - Each core keeps a replicated node-feature table in SBUF (fp16, 128-col
    256B slots, int16-addressable in two buckets) and gathers x_j
    feature-major with transposed SBUF-source dma_gather.
  - Message MLP: TensorE matmul per 128-edge tile (gathered tile is the
    stationary operand; edge_attr rows are appended below the features).
    LeakyReLU and the mean's 1/deg(dst) scale fuse into one ScalarE
    activation per tile.
  - Aggregation: per-tile one-hot fp8 selector matrices (host-built,
    streamed from HBM) matmul'd against messages, accumulating
    feature-major per-128-node-window sums in PSUM.
  - Update MLP per window; h_next is AllGather'd between layers to rebuild
    the table.
  - Global mean-pool via per-window batch-selector matmuls; final MLP runs
    replicated on every core.

Host-side work is integer index prep (sort/bincount/one-hot selectors) and
layout/dtype staging; all float compute runs on device.
"""

import numpy as np

DEBUG = False

import concourse.bass as bass
import concourse.bacc as bacc
import concourse.mybir as mybir
import concourse.tile as tile
from concourse.bass_utils import run_bass_kernel_spmd

F32 = mybir.dt.float32
F16 = mybir.dt.float16
F8 = mybir.dt.float8e4
I16 = mybir.dt.int16
AF = mybir.ActivationFunctionType
NP_F8 = mybir.dt.np(F8)

N_GRAPHS = 32
HID = [64, 128, 256]
MLP_DIMS = [256, 128, 64, 1]
N_CORES = 8

ELEM = 128      # fp16 feature slots per table entry (256 bytes)
WIN = 128       # nodes per aggregation window
GROUP_W = 4     # windows per gather group
LRELU = 0.2


def _cdiv(a, b):
    return -(-a // b)


_LRELU_OP = None


def _get_lrelu_op():
    """out = max(s*x, 0.2*s*x) in one DVE pass (s per-partition, 0.2 imm)."""
    global _LRELU_OP
    if _LRELU_OP is not None:
        return _LRELU_OP
    import concourse.dve_ops as dops
    from concourse.dve_spec import Spec, Src0, C0, C2, maxx
    name = "LRELU_SCALE_ANT"
    if name not in dops._SUB_OPCODE_FOR_NAME:
        row = max(dops._SUB_OPCODE_FOR_NAME.values()) + 1
        assert row < 0x20
        dops._SUB_OPCODE_FOR_NAME[name] = row
    spec = Spec(
        body=maxx(Src0 * C0, Src0 * C0 * C2),
        reference=lambda in0, in1, c0, c1, c2: np.maximum(
            in0 * c0, in0 * c0 * c2),
    )
    shas = {}
    for ver in ("v3", "v4"):
        try:
            probe = dops.DveOp(name, spec, subdim=False, uops_sha={})
            probe.compile(ver)
        except ValueError as ex:
            import re
            m = re.search(r"\{}: ([0-9a-f]{{16}})".format(ver), str(ex))
            if not m:
                m = re.search(r"([0-9a-f]{16}) \u2260|([0-9a-f]{16}) ", str(ex))
            shas[ver] = re.search(r"\(" + ver + r": ([0-9a-f]+)", str(ex)).group(1)
    op = dops.DveOp(name, spec, subdim=False, uops_sha=shas)
    if not any(o.name == name for o in dops.OPS):
        dops.OPS.append(op)
    dops.CUSTOM_DVE_SPECS[name] = spec
    _LRELU_OP = op
    return op


class Cfg:
    pass


# ============================================================ host index prep
def host_prep(inputs, n_cores=N_CORES):
    x = np.asarray(inputs["x"], np.float32)
    ei = np.asarray(inputs["edge_index"], np.int64)
    ea = np.asarray(inputs["edge_attr"], np.float32)
    batch = np.asarray(inputs["batch"], np.int64)

    n_nodes, node_dim = x.shape
    n_edges = ei.shape[1]

    cfg = Cfg()
    cfg.n_cores = n_cores
    cfg.n_nodes = n_nodes
    cfg.node_dim = node_dim
    cfg.n_graphs = N_GRAPHS
    npc = n_nodes // n_cores
    assert npc * n_cores == n_nodes
    cfg.npc = npc
    lo = min(_cdiv(_cdiv(npc, 2), 128) * 128, npc)
    hi = _cdiv(npc - lo, 128) * 128
    cfg.lo, cfg.hi = lo, hi
    cfg.hi_used = npc - lo
    cfg.lo_total = lo * n_cores
    cfg.hi_total = hi * n_cores
    cfg.slots = cfg.lo_total + cfg.hi_total
    cfg.stripes = cfg.slots // 128
    cfg.lo_stripes = cfg.lo_total // 128
    assert cfg.lo_total < 32768 and cfg.hi_total < 32768
    cfg.n_win = _cdiv(npc, WIN)
    cfg.last_win_nodes = npc - (cfg.n_win - 1) * WIN

    src = ei[0].astype(np.int64)
    dst = ei[1].astype(np.int64)
    deg = np.bincount(dst, minlength=n_nodes).astype(np.float32)
    inv_deg = (1.0 / np.maximum(deg, 1.0)).astype(np.float32)

    c_of = np.arange(n_nodes) // npc
    r_of = np.arange(n_nodes) % npc
    slot = np.where(
        r_of < lo,
        c_of * lo + r_of,
        cfg.lo_total + c_of * hi + (r_of - lo),
    ).astype(np.int64)
    slot_of_node = slot

    sslot = slot[src]
    ecore = dst // npc
    ewin = (dst % npc) // WIN
    ebuck = (sslot >= cfg.lo_total).astype(np.int64)

    key = (ecore * cfg.n_win + ewin) * 2 + ebuck
    cnt = np.bincount(key, minlength=n_cores * cfg.n_win * 2).reshape(
        n_cores, cfg.n_win, 2)
    T = np.maximum(_cdiv(cnt.max(axis=0), 128), 1)   # [n_win, 2]
    cfg.T = T
    cfg.n_tiles = int(T.sum())
    cfg.e_pad = cfg.n_tiles * 128

    groups = [list(range(g, min(g + GROUP_W, cfg.n_win)))
              for g in range(0, cfg.n_win, GROUP_W)]
    cfg.groups = groups

    # padded stream order: per group: [A segs of its windows] [B segs]
    seg_off = {}
    pos = 0
    for ws in groups:
        for b in (0, 1):
            for w in ws:
                seg_off[(w, b)] = pos
                pos += int(T[w, b]) * 128
    assert pos == cfg.e_pad
    cfg.seg_off = seg_off

    order = np.lexsort((ebuck, ewin, ecore))
    src_s = sslot[order]
    dst_s = dst[order]
    ea_s = ea[order]
    inv_s = inv_deg[dst[order]]

    ck = (ecore[order] * cfg.n_win + ewin[order]) * 2 + ebuck[order]
    seg_starts = np.searchsorted(ck, np.arange(n_cores * cfg.n_win * 2))
    seg_ends = np.append(seg_starts[1:], n_edges)

    win_pad = cfg.n_win * WIN
    e_pad = cfg.e_pad
    in_maps = []
    wts = _pack_weights(inputs, node_dim)
    ident = np.eye(128, dtype=np.float16)

    for c in range(n_cores):
        g_idx = np.zeros(e_pad, np.int64)
        buck_flag = np.zeros(e_pad, np.bool_)
        e_a = np.zeros((4, e_pad), np.float32)
        invd = np.zeros(e_pad, np.float32)
        selcol = np.full(e_pad, -1, np.int64)

        for w in range(cfg.n_win):
            for b in (0, 1):
                s0 = seg_starts[(c * cfg.n_win + w) * 2 + b]
                s1 = seg_ends[(c * cfg.n_win + w) * 2 + b]
                n = s1 - s0
                o = seg_off[(w, b)]
                assert n <= T[w, b] * 128
                if n:
                    buck_flag[o:o + n] = bool(b)
                    g_idx[o:o + n] = src_s[s0:s1] - (cfg.lo_total if b else 0)
                    e_a[:3, o:o + n] = ea_s[s0:s1].T
                    e_a[3, o:o + n] = 1.0
                    invd[o:o + n] = inv_s[s0:s1]
                    selcol[o:o + n] = (dst_s[s0:s1] % npc) - w * WIN

        gi = np.zeros((128, e_pad // 16), np.int16)
        base = g_idx.astype(np.int16).reshape(-1, 16).T
        for k in range(8):
            gi[16 * k:16 * k + 16] = base

        # layer-0 edge stream: [x[src](10) | ea(3) | 1] fp16, feature-major
        xe = np.zeros((node_dim + 4, e_pad), np.float16)
        edge_valid = selcol >= 0
        # recover per-edge src x via the slot->x map
        xe[:node_dim, :] = 0.0

        sel = np.zeros((128, cfg.n_tiles * 128), np.uint8)
        tt = np.arange(e_pad) // 128
        ee = np.arange(e_pad) % 128
        m = selcol >= 0
        sel[ee[m], tt[m] * 128 + selcol[m]] = 0x38

        xt = np.zeros((node_dim, win_pad), np.float16)
        xt[:, :npc] = x[c * npc:(c + 1) * npc].astype(np.float16).T

        sb = np.zeros((128, cfg.n_win * N_GRAPHS), np.uint8)
        bl = batch[c * npc:(c + 1) * npc].astype(np.int64)
        pp = np.arange(npc) % WIN
        ww = np.arange(npc) // WIN
        sb[pp, ww * N_GRAPHS + bl] = 0x38

        xsrc_slot = np.zeros((cfg.slots, node_dim), np.float16)
        xsrc_slot[slot_of_node] = x.astype(np.float16)
        gsl = g_idx + np.where(buck_flag, cfg.lo_total, 0)
        xe[:node_dim, :] = xsrc_slot[gsl].T
        xe[node_dim:node_dim + 4, :] = e_a.astype(np.float16)
        xe[:, ~edge_valid] = 0.0

        m_ = {
            "xeT": xe,
            "gidx": gi,
            "eaT": e_a.astype(np.float16),
            "invd": invd.reshape(-1, 128).T.astype(np.float32).copy(),
            "sel": sel.view(NP_F8),
            "xT_loc": xt,
            "selB": sb.view(NP_F8),
            "ident": ident,
        }
        m_.update(wts)
        in_maps.append(m_)
    return cfg, in_maps


def _pack_weights(inputs, node_dim):
    wts = {}
    node_in = node_dim
    for li in range(len(HID)):
        mw = np.asarray(inputs[f"mw{li}"], np.float32)
        mb = np.asarray(inputs[f"mb{li}"], np.float32)
        wts[f"mwp{li}"] = np.concatenate(
            [mw, mb[None, :]], axis=0).astype(np.float16)
        wts[f"uw{li}"] = np.asarray(inputs[f"uw{li}"], np.float16)
        wts[f"ub{li}"] = np.asarray(inputs[f"ub{li}"], np.float16)[None, :]
        node_in = HID[li]
    for li in range(len(MLP_DIMS) - 1):
        wts[f"fw{li}"] = np.asarray(inputs[f"fw{li}"], np.float16)
        wts[f"fb{li}"] = np.asarray(
            inputs[f"fb{li}"], np.float32).reshape(-1, 1)
    return wts


# =============================================================== bass builder
def build_program(cfg):
    nc = bacc.Bacc(
        "TRN2",
        target_bir_lowering=False,
        debug=False,
        enable_asserts=False,
        num_devices=cfg.n_cores,
    )
    n_win, npc, n_tiles, e_pad = cfg.n_win, cfg.npc, cfg.n_tiles, cfg.e_pad
    slots, stripes, lo_stripes = cfg.slots, cfg.stripes, cfg.lo_stripes
    win_pad = n_win * WIN
    NG = cfg.n_graphs
    T = cfg.T
    groups = cfg.groups
    seg_off = cfg.seg_off
    core_ids = list(range(cfg.n_cores))

    D = {}

    def din(name, shape, dt):
        D[name] = nc.dram_tensor(name, list(shape), dt, kind="ExternalInput")

    din("xeT", (cfg.node_dim + 4, e_pad), F16)
    din("gidx", (128, e_pad // 16), I16)
    din("eaT", (4, e_pad), F16)
    din("invd", (128, n_tiles), F32)
    din("sel", (128, n_tiles * 128), F8)
    din("xT_loc", (cfg.node_dim, win_pad), F16)
    din("selB", (128, n_win * NG), F8)
    din("ident", (128, 128), F16)
    node_in = cfg.node_dim
    for li, dout in enumerate(HID):
        din(f"mwp{li}", (node_in + 4, dout), F16)
        din(f"uw{li}", (dout + node_in, dout), F16)
        din(f"ub{li}", (1, dout), F16)
        node_in = dout
    for li in range(len(MLP_DIMS) - 1):
        din(f"fw{li}", (MLP_DIMS[li], MLP_DIMS[li + 1]), F16)
        din(f"fb{li}", (MLP_DIMS[li + 1], 1), F32)
    out_t = nc.dram_tensor("out", [NG, 1], F32, kind="ExternalOutput")

    # group extents in the padded stream
    g_meta = []
    for ws in groups:
        nA = int(sum(T[w, 0] for w in ws)) * 128
        nB = int(sum(T[w, 1] for w in ws)) * 128
        g_meta.append((seg_off[(ws[0], 0)], nA, nB))
    max_g_cols = max(nA + nB for _, nA, nB in g_meta)
    max_w_cols = int((T[:, 0] + T[:, 1]).max()) * 128

    from contextlib import ExitStack
    with ExitStack() as _es:
        tc = _es.enter_context(tile.TileContext(nc))
        p_table = _es.enter_context(tc.tile_pool(name="table", bufs=1))
        p_res = _es.enter_context(tc.tile_pool(name="res", bufs=1))
        p_wts = _es.enter_context(tc.tile_pool(name="wts", bufs=1))
        p_gath = _es.enter_context(tc.tile_pool(name="gath", bufs=2))
        p_sel = _es.enter_context(tc.tile_pool(name="selp", bufs=2))
        p_gix = _es.enter_context(tc.tile_pool(name="gix", bufs=2))
        p_ivd = _es.enter_context(tc.tile_pool(name="ivd", bufs=2))
        p_ea = _es.enter_context(tc.tile_pool(name="eal2", bufs=2))
        p_msg = _es.enter_context(tc.tile_pool(name="msg", bufs=6))
        p_aggs = _es.enter_context(tc.tile_pool(name="aggs", bufs=4))
        p_hloc = _es.enter_context(tc.tile_pool(name="hloc", bufs=2))
        p_hn = _es.enter_context(tc.tile_pool(name="hnext", bufs=3))
        p_small = _es.enter_context(tc.tile_pool(name="small", bufs=1))
        pp_msg = _es.enter_context(tc.tile_pool(name="pmsg", bufs=3, space="PSUM"))
        pp_agg = _es.enter_context(tc.tile_pool(name="pagg", bufs=2, space="PSUM"))
        pp_upd = _es.enter_context(tc.tile_pool(name="pupd", bufs=2, space="PSUM"))
        pp_pool = _es.enter_context(tc.tile_pool(name="ppool", bufs=1, space="PSUM"))
        p_dram = _es.enter_context(tc.tile_pool(name="dram", bufs=1, space="DRAM"))
        if True:
            lo_s = lo_stripes
            hi_s = stripes - lo_stripes
            tableA = p_table.tile([128, lo_s * ELEM], F16, tag="tabA")
            tableB = p_table.tile([128, hi_s * ELEM], F16, tag="tabB")
            tblA3 = tableA[:].rearrange("p (s c) -> p s c", c=ELEM)
            tblB3 = tableB[:].rearrange("p (s c) -> p s c", c=ELEM)
            selB_sb = p_res.tile([128, n_win * NG], F8, tag="selB")
            ident_sb = p_res.tile([128, 128], F16, tag="ident")
            ones_row = p_res.tile([1, 128], F16, tag="ones_r")
            ones_col = p_res.tile([128, 1], F16, tag="ones_c")

            nc.sync.dma_start(selB_sb[:], D["selB"][:])
            nc.sync.dma_start(ident_sb[:], D["ident"][:])
            nc.vector.memset(ones_row[:], 1.0)
            nc.vector.memset(ones_col[:], 1.0)

            # weights -> SBUF (pre-cast to fp16 on host; HWDGE loads keep the
            # Pool queue free for gathers/collectives)
            W = {}
            node_in = cfg.node_dim
            for li, dout in enumerate(HID):
                mw_chunks = []
                for k, r in enumerate(range(0, node_in + 4, 128)):
                    r1 = min(r + 128, node_in + 4)
                    t = p_wts.tile([r1 - r, dout], F16, tag=f"mwp{li}_{k}")
                    nc.scalar.dma_start(t[:], D[f"mwp{li}"][r:r1, :])
                    mw_chunks.append(t)
                W[f"mwp{li}"] = mw_chunks
                # uw chunks: agg rows [0:dout] in 128-chunks, then h rows
                chunks = []
                for r in list(range(0, dout, 128)):
                    chunks.append((r, min(r + 128, dout)))
                for r in list(range(0, node_in, 128)):
                    chunks.append((dout + r, dout + min(r + 128, node_in)))
                uws = []
                for k, (r0, r1) in enumerate(chunks):
                    t = p_wts.tile([r1 - r0, dout], F16, tag=f"uw{li}_{k}")
                    nc.scalar.dma_start(t[:], D[f"uw{li}"][r0:r1, :])
                    uws.append(t)
                W[f"uw{li}"] = uws
                t = p_wts.tile([1, dout], F16, tag=f"ub{li}")
                nc.scalar.dma_start(t[:], D[f"ub{li}"][:])
                W[f"ub{li}"] = t
                node_in = dout
            for li in range(len(MLP_DIMS) - 1):
                fws = []
                for k, r in enumerate(range(0, MLP_DIMS[li], 128)):
                    r1 = min(r + 128, MLP_DIMS[li])
                    t = p_wts.tile([r1 - r, MLP_DIMS[li + 1]], F16,
                                   tag=f"fw{li}_{k}")
                    nc.scalar.dma_start(t[:], D[f"fw{li}"][r:r1, :])
                    fws.append(t)
                W[f"fw{li}"] = fws
                t = p_wts.tile([MLP_DIMS[li + 1], 1], F32, tag=f"fb{li}")
                nc.sync.dma_start(t[:], D[f"fb{li}"][:])
                W[f"fb{li}"] = t

            hlocT = p_hloc.tile([cfg.node_dim, win_pad], F16, tag="hloc")
            nc.sync.dma_start(hlocT[:], D["xT_loc"][:])
            # table init off the Pool queue (DVE+ScalarE run idle at startup;
            # Pool must stay free for gathers + collectives). Only cols
            # dout:ELEM are ever read uninitialized-as-garbage, and only as
            # untouched DMA payload.
            nc.vector.memset(tableA[:], 0.0)
            nc.vector.memset(tableB[:], 0.0)
            zeros8 = p_res.tile([128, max(HID)], F8, tag="zeros")
            nc.vector.memset(zeros8[:], 0.0)

            lo_w = cfg.lo // 128
            assert cfg.hi_total > 0
            hi_nw = n_win - lo_w
            amid = max(1, lo_w // 2)
            hmid = lo_w + max(1, hi_nw // 2)
            chunk_bounds = [(0, amid), (amid, lo_w),
                            (lo_w, min(hmid, n_win)),
                            (min(hmid, n_win), n_win)]
            chunk_bounds = [(a, b) for a, b in chunk_bounds if b > a]
            chunk_of_w = {}
            for ci, (a, b) in enumerate(chunk_bounds):
                for w in range(a, b):
                    chunk_of_w[w] = ci
            ag_in = {}
            ag_out = {}
            for li in (0, 1):
                for ci, (a, b) in enumerate(chunk_bounds):
                    ag_in[(li, ci)] = p_dram.tile(
                        [128, (b - a) * HID[li]], F8,
                        tag=f"agi{li}_{ci}", name=f"agi{li}_{ci}")
                    ag_out[(li, ci)] = p_dram.tile(
                        [cfg.n_cores * 128, (b - a) * HID[li]], F8,
                        tag=f"ago{li}_{ci}", name=f"ago{li}_{ci}",
                        addr_space="Shared")
            FP = HID[2] + 1
            gp_in = p_dram.tile([NG, FP], F32, tag="gpi")
            if DEBUG:
                dbg_h3 = p_dram.tile([128, n_win * HID[2]], F16, tag="dbgh3",
                                     name="dbg_h3")
                dbg_ag = p_dram.tile([128, n_win * HID[2]], F16, tag="dbgag",
                                     name="dbg_ag")
                dbg_msg = p_dram.tile([128, 16 * HID[2]], F16, tag="dbgmsg",
                                      name="dbg_msg")
            gp_out = p_dram.tile([cfg.n_cores * NG, FP], F32, tag="gpo", addr_space="Shared")

            psum_pool = pp_pool.tile([NG, FP], F32)

            lo_s_pc = cfg.lo // 128
            hi_s_pc = n_win - lo_s_pc

            def _ag_emit(li, ci, dout):
                nc.gpsimd.collective_compute(
                    "AllGather",
                    mybir.AluOpType.bypass,
                    replica_groups=[core_ids],
                    ins=[ag_in[(li, ci)].opt()],
                    outs=[ag_out[(li, ci)].opt()],
                )

            def _rebuild_emit(li, dout):
                # deferred: overwrites the tables, so must come after the
                # layer's last gather (Tile WAR-orders it automatically).
                # gpsimd (SWDGE) DMA casts the fp8 gathered payload to the
                # fp16 table format.
                for ci, (a, b) in enumerate(chunk_bounds):
                    nw = b - a
                    for rr in range(cfg.n_cores):
                        frag = ag_out[(li, ci)][rr * 128:(rr + 1) * 128, :]\
                            .rearrange("p (s c) -> p s c", c=dout)
                        if b <= lo_w:
                            s0 = rr * lo_s_pc + a
                            nc.gpsimd.dma_start(
                                tblA3[:, s0:s0 + nw, 0:dout], frag[:])
                        else:
                            s0 = rr * hi_s_pc + (a - lo_w)
                            nc.gpsimd.dma_start(
                                tblB3[:, s0:s0 + nw, 0:dout], frag[:])

            # ========================= layers
            node_in = cfg.node_dim
            for li, dout in enumerate(HID):
                mwp = W[f"mwp{li}"]
                nch = _cdiv(dout, 128)
                last = li == len(HID) - 1
                hnextT = None
                if not last:
                    hnextT = p_hloc.tile([dout, win_pad], F16, tag="hloc")

                for gi_, ws in enumerate(groups):
                    start, nA, nB = g_meta[gi_]
                    ncols = nA + nB
                    selbuf = p_sel.tile([128, max_g_cols], F8, tag="selp")
                    ivdb = p_ivd.tile([128, max_g_cols // 128], F32, tag="ivd")
                    nc.sync.dma_start(
                        ivdb[:, 0:ncols // 128],
                        D["invd"][:, start // 128:(start + ncols) // 128])
                    nc.sync.dma_start(
                        selbuf[:, 0:ncols],
                        D["sel"][:, start:start + ncols])
                    if li == 0:
                        gbuf = p_gath.tile([cfg.node_dim + 4, max_g_cols],
                                           F16, tag="gath", name="gbuf")
                        nc.sync.dma_start(
                            gbuf[:, 0:ncols],
                            D["xeT"][:, start:start + ncols])
                    else:
                        gbuf = p_gath.tile([128, max_g_cols], F16, tag="gath",
                                           name="gbuf")
                        gixb = p_gix.tile([128, max_g_cols // 16], I16,
                                          tag="gix")
                        nc.sync.dma_start(
                            gixb[:, 0:ncols // 16],
                            D["gidx"][:, start // 16:(start + ncols) // 16])
                        for b, coff, n_b in ((0, 0, nA), (1, nA, nB)):
                            if n_b == 0:
                                continue
                            tb = tableA[:] if b == 0 else tableB[:]
                            nc.gpsimd.dma_gather(
                                gbuf[:, coff:coff + n_b].rearrange(
                                    "p (o n) -> p o n", o=1),
                                tb,
                                gixb[:, coff // 16:(coff + n_b) // 16],
                                num_idxs=n_b,
                                num_idxs_reg=n_b,
                                elem_size=ELEM,
                                transpose=True,
                                single_packet=False,
                                sbuf_tokens_per_rank=128,
                                sbuf_free_dim_per_rank=ELEM * 2,
                            )
                        if node_in + 4 <= 128:
                            nc.sync.dma_start(
                                gbuf[node_in:node_in + 4, 0:ncols],
                                D["eaT"][:, start:start + ncols])

                    # window-major processing; stream offsets are bucket-major
                    aoff = 0
                    boff = nA
                    for w in ws:
                        t0, t1 = int(T[w, 0]), int(T[w, 1])
                        wcols = []
                        for k in range(t0):
                            wcols.append(aoff + k * 128)
                        for k in range(t1):
                            wcols.append(boff + k * 128)
                        aoff += t0 * 128
                        boff += t1 * 128

                        eabuf = None
                        if node_in + 4 > 128:
                            eabuf = p_ea.tile([4, max_w_cols], F16, tag="eal2")
                            nc.sync.dma_start(
                                eabuf[:, 0:t0 * 128],
                                D["eaT"][:, start + wcols[0]:
                                         start + wcols[0] + t0 * 128])
                            nc.sync.dma_start(
                                eabuf[:, t0 * 128:(t0 + t1) * 128],
                                D["eaT"][:, start + wcols[t0]:
                                         start + wcols[t0] + t1 * 128])

                        pagg = pp_agg.tile([128, nch * 128], F32, tag="pagg")
                        n_wt = len(wcols)
                        pend = None

                        def _sel_emit(pk, pcol, pmsg_t):
                            for ch in range(nch):
                                c0, c1 = ch * 128, min(dout, ch * 128 + 128)
                                nc.tensor.matmul(
                                    pagg[0:c1 - c0, ch * 128:ch * 128 + 128],
                                    pmsg_t[:, c0:c1],
                                    selbuf[:, pcol:pcol + 128],
                                    start=(pk == 0 and ch == 0),
                                    stop=(pk == n_wt - 1 and ch == nch - 1),
                                    skip_group_check=True)

                        for k, col in enumerate(wcols):
                            pmsg = pp_msg.tile([128, dout], F32, tag="pmsg")
                            if node_in + 4 <= 128:
                                nc.tensor.matmul(
                                    pmsg[:], gbuf[0:node_in + 4, col:col + 128],
                                    mwp[0][:], start=True, stop=True)
                            else:
                                nc.tensor.matmul(
                                    pmsg[:], gbuf[0:128, col:col + 128],
                                    mwp[0][:], start=True, stop=False)
                                nc.tensor.matmul(
                                    pmsg[:], eabuf[:, k * 128:k * 128 + 128],
                                    mwp[1][:], start=False, stop=True)
                            msg = p_msg.tile([128, dout], F16, tag="msg")
                            tloc = col // 128
                            sc_ap = ivdb[:, tloc:tloc + 1]
                            if k % 2 == 0:
                                nc.scalar.activation(
                                    msg[:], pmsg[:], AF.Prelu,
                                    scale=sc_ap, alpha=LRELU)
                            else:
                                nc.vector._custom_dve(
                                    _get_lrelu_op(), out=msg[:],
                                    in0=pmsg[:], s0=sc_ap, imm2=LRELU)
                            if DEBUG and last and w == 0 and k < 16:
                                nc.sync.dma_start(
                                    dbg_msg[:, k * dout:(k + 1) * dout],
                                    msg[:])
                            if pend is not None:
                                _sel_emit(*pend)
                            pend = (k, col, msg)
                        if pend is not None:
                            _sel_emit(*pend)

                        # ----- window update
                        wn = WIN if w < n_win - 1 else cfg.last_win_nodes
                        lhs_list = []
                        for ch in range(nch):
                            c0, c1 = ch * 128, min(dout, ch * 128 + 128)
                            a = p_aggs.tile([128, 128], F16, tag="aggs")
                            nc.vector.tensor_copy(
                                a[0:c1 - c0, :],
                                pagg[0:c1 - c0, ch * 128:ch * 128 + 128])
                            lhs_list.append(a[0:c1 - c0, :])
                        for r in range(0, node_in, 128):
                            r1 = min(node_in, r + 128)
                            lhs_list.append(
                                hlocT[r:r1, w * WIN:w * WIN + 128])
                        uws = W[f"uw{li}"]
                        assert len(uws) == len(lhs_list)
                        pupd = pp_upd.tile([128, max(HID)], F32, tag="pupd")
                        for i, lhs in enumerate(lhs_list):
                            nc.tensor.matmul(
                                pupd[:, 0:dout], lhs, uws[i][:],
                                start=(i == 0), stop=False)
                        nc.tensor.matmul(
                            pupd[:, 0:dout], ones_row[:], W[f"ub{li}"][:],
                            start=False, stop=True)
                        hn = p_hn.tile([128, max(HID)], F16, tag="hnext")
                        nc.scalar.activation(
                            hn[:, 0:dout], pupd[:, 0:dout], AF.Prelu,
                            alpha=LRELU)
                        if not last:
                            hn8 = p_hn.tile([128, max(HID)], F8, tag="hn8")
                            nc.vector.tensor_copy(
                                hn8[:, 0:dout], hn[:, 0:dout])
                            ci = chunk_of_w[w]
                            wl = w - chunk_bounds[ci][0]
                            agi = ag_in[(li, ci)]
                            nc.sync.dma_start(
                                agi[:, wl * dout:wl * dout + dout]
                                if wn == WIN else
                                agi[0:wn, wl * dout:wl * dout + dout],
                                hn8[:, 0:dout] if wn == WIN
                                else hn8[0:wn, 0:dout])
                            if wn < WIN:
                                nc.sync.dma_start(
                                    agi[wn:128,
                                        wl * dout:wl * dout + dout],
                                    zeros8[wn:128, 0:dout])
                            for ch in range(nch):
                                c0, c1 = ch * 128, min(dout, ch * 128 + 128)
                                pt = pp_upd.tile([128, 128], F16, tag="pupd")
                                nc.tensor.transpose(
                                    pt[0:c1 - c0, :], hn[:, c0:c1],
                                    ident_sb[:])
                                nc.vector.tensor_copy(
                                    hnextT[c0:c1, w * WIN:w * WIN + 128],
                                    pt[0:c1 - c0, :])
                            if w == chunk_bounds[ci][1] - 1:
                                _ag_emit(li, ci, dout)
                        else:
                            if DEBUG:
                                nc.sync.dma_start(
                                    dbg_h3[:, w * dout:(w + 1) * dout],
                                    hn[:, 0:dout])
                                for _ch in range(nch):
                                    _cw = (min(dout, _ch * 128 + 128)
                                           - _ch * 128)
                                    nc.sync.dma_start(
                                        dbg_ag[0:_cw,
                                               (w * 2 + _ch) * 128:
                                               (w * 2 + _ch) * 128 + 128],
                                        lhs_list[_ch])
                            nc.tensor.matmul(
                                psum_pool[:, 0:dout],
                                selB_sb[:, w * NG:(w + 1) * NG],
                                hn[:, 0:dout],
                                start=(w == 0), stop=False,
                                skip_group_check=True)
                            nc.tensor.matmul(
                                psum_pool[:, dout:dout + 1],
                                selB_sb[:, w * NG:(w + 1) * NG],
                                ones_col[:],
                                start=False, stop=(w == n_win - 1),
                                skip_group_check=True)

                # ----- end of layer
                if not last:
                    _rebuild_emit(li, dout)
                    hlocT = hnextT
                node_in = dout

            # ========================= pooling finale + MLP (replicated)
            gp_sb = p_gath.tile([NG, FP], F32, tag="gath", name="gp_sb")
            nc.vector.tensor_copy(gp_sb[:], psum_pool[:])
            nc.sync.dma_start(gp_in[:], gp_sb[:])
            nc.gpsimd.collective_compute(
                "AllGather",
                mybir.AluOpType.bypass,
                replica_groups=[core_ids],
                ins=[gp_in.opt()],
                outs=[gp_out.opt()],
            )
            gall = p_gath.tile([NG, FP * cfg.n_cores], F32, tag="gath", name="gall")
            nc.sync.dma_start(
                gall[:].rearrange("g (f r) -> g f r", r=cfg.n_cores),
                gp_out[:].rearrange("(r g) f -> g f r", g=NG))
            gsum = p_gath.tile([NG, FP], F32, tag="gath", name="gsum")
            nc.vector.tensor_reduce(
                gsum[:],
                gall[:].rearrange("g (f r) -> g f r", r=cfg.n_cores),
                axis=mybir.AxisListType.X,
                op=mybir.AluOpType.add)
            rec = p_small.tile([NG, 1], F32, tag="rec")
            nc.vector.reciprocal(rec[:], gsum[:, HID[2]:HID[2] + 1])
            g16 = p_small.tile([NG, HID[2]], F16, tag="g16")
            nc.scalar.activation(g16[:], gsum[:, 0:HID[2]], AF.Identity,
                                 scale=rec[:])
            gT = p_small.tile([128, 2 * NG], F16, tag="gT")
            for ch in range(2):
                pt = pp_upd.tile([128, 128], F16, tag="pupd")
                nc.tensor.transpose(
                    pt[0:128, 0:NG], g16[:, ch * 128:(ch + 1) * 128],
                    ident_sb[0:NG, 0:NG])
                nc.vector.tensor_copy(
                    gT[:, ch * NG:(ch + 1) * NG], pt[0:128, 0:NG])
            pf1 = pp_upd.tile([128, NG], F32, tag="pupd")
            for ch in range(2):
                nc.tensor.matmul(
                    pf1[0:MLP_DIMS[1], :], W["fw0"][ch][:],
                    gT[:, ch * NG:(ch + 1) * NG],
                    start=(ch == 0), stop=(ch == 1))
            f1 = p_small.tile([MLP_DIMS[1], NG], F16, tag="f1")
            nc.scalar.activation(f1[:], pf1[0:MLP_DIMS[1], :], AF.Prelu,
                                 bias=W["fb0"][:], alpha=LRELU)
            pf2 = pp_upd.tile([128, NG], F32, tag="pupd")
            nc.tensor.matmul(pf2[0:MLP_DIMS[2], :], W["fw1"][0][:], f1[:],
                             start=True, stop=True)
            f2 = p_small.tile([MLP_DIMS[2], NG], F16, tag="f2")
            nc.scalar.activation(f2[:], pf2[0:MLP_DIMS[2], :], AF.Prelu,
                                 bias=W["fb1"][:], alpha=LRELU)
            pf3 = pp_upd.tile([128, NG], F32, tag="pupd")
            nc.tensor.matmul(pf3[0:1, :], W["fw2"][0][:], f2[:],
                             start=True, stop=True)
            fout = p_small.tile([1, NG], F32, tag="fo")
            nc.scalar.activation(fout[:], pf3[0:1, :], AF.Identity,
                                 bias=W["fb2"][:])
            nc.sync.dma_start(out_t[:].rearrange("g o -> o g"), fout[:])

    return nc


def kernel(**inputs):
    cfg, in_maps = host_prep(inputs)
    nc = build_program(cfg)
    nc.compile()
    res = run_bass_kernel_spmd(nc, in_maps, core_ids=list(range(cfg.n_cores)))
    return np.asarray(res.results[0]["out"], np.float32)



# revision 49
# speedup vs baseline: 1.2605x; 1.2231x over previous
"""GNN message-passing discriminator on 8 trn2 NeuronCores.

Strategy (edge-parallel by *destination* node):
  - Nodes sharded npc=6250/core; each edge lives on the core owning its dst.
  - Each core keeps a replicated node-feature table in SBUF (fp16, 128-col
    256B slots, int16-addressable in two buckets) and gathers x_j
    feature-major with transposed SBUF-source dma_gather.
  - Message MLP: TensorE matmul per 128-edge tile (gathered tile is the
    stationary operand; edge_attr rows are appended below the features).
    LeakyReLU and the mean's 1/deg(dst) scale fuse into one ScalarE
    activation per tile.
  - Aggregation: per-tile one-hot fp8 selector matrices (host-built,
    streamed from HBM) matmul'd against messages, accumulating
    feature-major per-128-node-window sums in PSUM.
  - Update MLP per window; h_next is AllGather'd between layers to rebuild
    the table.
  - Global mean-pool via per-window batch-selector matmuls; final MLP runs
    replicated on every core.

Host-side work is integer index prep (sort/bincount/one-hot selectors) and
layout/dtype staging; all float compute runs on device.
"""

import numpy as np

DEBUG = False

import concourse.bass as bass
import concourse.bacc as bacc
import concourse.mybir as mybir
import concourse.tile as tile
from concourse.bass_utils import run_bass_kernel_spmd

F32 = mybir.dt.float32
F16 = mybir.dt.float16
F8 = mybir.dt.float8e4
I16 = mybir.dt.int16
AF = mybir.ActivationFunctionType
NP_F8 = mybir.dt.np(F8)

N_GRAPHS = 32
HID = [64, 128, 256]
MLP_DIMS = [256, 128, 64, 1]
N_CORES = 8

ELEM = 128      # fp16 feature slots per table entry (256 bytes)
WIN = 128       # nodes per aggregation window
GROUP_W = 3     # windows per gather group
LRELU = 0.2


def _cdiv(a, b):
    return -(-a // b)


_LRELU_OP = None


def _get_lrelu_op():
    """out = max(s*x, 0.2*s*x) in one DVE pass (s per-partition, 0.2 imm)."""
    global _LRELU_OP
    if _LRELU_OP is not None:
        return _LRELU_OP
    import concourse.dve_ops as dops
    from concourse.dve_spec import Spec, Src0, C0, C2, maxx
    name = "LRELU_SCALE_ANT"
    if name not in dops._SUB_OPCODE_FOR_NAME:
        row = max(dops._SUB_OPCODE_FOR_NAME.values()) + 1
        assert row < 0x20
        dops._SUB_OPCODE_FOR_NAME[name] = row
    spec = Spec(
        body=maxx(Src0 * C0, Src0 * C0 * C2),
        reference=lambda in0, in1, c0, c1, c2: np.maximum(
            in0 * c0, in0 * c0 * c2),
    )
    shas = {}
    for ver in ("v3", "v4"):
        try:
            probe = dops.DveOp(name, spec, subdim=False, uops_sha={})
            probe.compile(ver)
        except ValueError as ex:
            import re
            m = re.search(r"\{}: ([0-9a-f]{{16}})".format(ver), str(ex))
            if not m:
                m = re.search(r"([0-9a-f]{16}) \u2260|([0-9a-f]{16}) ", str(ex))
            shas[ver] = re.search(r"\(" + ver + r": ([0-9a-f]+)", str(ex)).group(1)
    op = dops.DveOp(name, spec, subdim=False, uops_sha=shas)
    if not any(o.name == name for o in dops.OPS):
        dops.OPS.append(op)
    dops.CUSTOM_DVE_SPECS[name] = spec
    _LRELU_OP = op
    return op


class Cfg:
    pass


# ============================================================ host index prep
def host_prep(inputs, n_cores=N_CORES):
    x = np.asarray(inputs["x"], np.float32)
    ei = np.asarray(inputs["edge_index"], np.int64)
    ea = np.asarray(inputs["edge_attr"], np.float32)
    batch = np.asarray(inputs["batch"], np.int64)

    n_nodes, node_dim = x.shape
    n_edges = ei.shape[1]

    cfg = Cfg()
    cfg.n_cores = n_cores
    cfg.n_nodes = n_nodes
    cfg.node_dim = node_dim
    cfg.n_graphs = N_GRAPHS
    npc = n_nodes // n_cores
    assert npc * n_cores == n_nodes
    cfg.npc = npc
    lo = min(_cdiv(_cdiv(npc, 2), 128) * 128, npc)
    hi = _cdiv(npc - lo, 128) * 128
    cfg.lo, cfg.hi = lo, hi
    cfg.hi_used = npc - lo
    cfg.lo_total = lo * n_cores
    cfg.hi_total = hi * n_cores
    cfg.slots = cfg.lo_total + cfg.hi_total
    cfg.stripes = cfg.slots // 128
    cfg.lo_stripes = cfg.lo_total // 128
    assert cfg.lo_total < 32768 and cfg.hi_total < 32768
    cfg.n_win = _cdiv(npc, WIN)
    cfg.last_win_nodes = npc - (cfg.n_win - 1) * WIN

    src = ei[0].astype(np.int64)
    dst = ei[1].astype(np.int64)
    deg = np.bincount(dst, minlength=n_nodes).astype(np.float32)
    inv_deg = (1.0 / np.maximum(deg, 1.0)).astype(np.float32)

    # per-core node->window assignment balancing per-window edge load (LPT):
    # shrinks the max-over-cores tile padding in the per-(window,bucket)
    # edge stream
    import heapq
    pos_of = np.empty(n_nodes, np.int64)
    caps_t = np.full(cfg.n_win, WIN, np.int64)
    caps_t[-1] = cfg.last_win_nodes
    for c in range(n_cores):
        nodes = np.arange(c * npc, (c + 1) * npc)
        d = deg[nodes]
        order_ = np.argsort(-d, kind="stable")
        counts = np.zeros(cfg.n_win, np.int64)
        pos_local = np.empty(npc, np.int64)
        heap = [(0.0, w) for w in range(cfg.n_win)]
        heapq.heapify(heap)
        for idx in order_:
            spill = []
            while True:
                load, w = heapq.heappop(heap)
                if counts[w] < caps_t[w]:
                    break
                spill.append((load, w))
            for it in spill:
                heapq.heappush(heap, it)
            pos_local[idx] = w * WIN + counts[w]
            counts[w] += 1
            heapq.heappush(heap, (load + float(d[idx]), w))
        pos_of[nodes] = pos_local

    c_of = np.arange(n_nodes) // npc
    r_of = pos_of
    slot = np.where(
        r_of < lo,
        c_of * lo + r_of,
        cfg.lo_total + c_of * hi + (r_of - lo),
    ).astype(np.int64)
    slot_of_node = slot

    sslot = slot[src]
    ecore = dst // npc
    ewin = pos_of[dst] // WIN
    ebuck = (sslot >= cfg.lo_total).astype(np.int64)

    key = (ecore * cfg.n_win + ewin) * 2 + ebuck
    cnt = np.bincount(key, minlength=n_cores * cfg.n_win * 2).reshape(
        n_cores, cfg.n_win, 2)
    T = np.maximum(_cdiv(cnt.max(axis=0), 128), 1)   # [n_win, 2]
    cfg.T = T
    cfg.n_tiles = int(T.sum())
    cfg.e_pad = cfg.n_tiles * 128

    groups = [list(range(g, min(g + GROUP_W, cfg.n_win)))
              for g in range(0, cfg.n_win, GROUP_W)]
    cfg.groups = groups

    # padded stream order: per group: [A segs of its windows] [B segs]
    seg_off = {}
    pos = 0
    for ws in groups:
        for b in (0, 1):
            for w in ws:
                seg_off[(w, b)] = pos
                pos += int(T[w, b]) * 128
    assert pos == cfg.e_pad
    cfg.seg_off = seg_off

    order = np.lexsort((ebuck, ewin, ecore))
    src_s = sslot[order]
    dst_s = dst[order]
    ea_s = ea[order]
    inv_s = inv_deg[dst[order]]

    ck = (ecore[order] * cfg.n_win + ewin[order]) * 2 + ebuck[order]
    seg_starts = np.searchsorted(ck, np.arange(n_cores * cfg.n_win * 2))
    seg_ends = np.append(seg_starts[1:], n_edges)

    win_pad = cfg.n_win * WIN
    e_pad = cfg.e_pad
    in_maps = []
    wts = _pack_weights(inputs, node_dim)
    ident = np.eye(128, dtype=np.float16)

    for c in range(n_cores):
        g_idx = np.zeros(e_pad, np.int64)
        buck_flag = np.zeros(e_pad, np.bool_)
        e_a = np.zeros((4, e_pad), np.float32)
        invd = np.zeros(e_pad, np.float32)
        selcol = np.full(e_pad, -1, np.int64)

        for w in range(cfg.n_win):
            for b in (0, 1):
                s0 = seg_starts[(c * cfg.n_win + w) * 2 + b]
                s1 = seg_ends[(c * cfg.n_win + w) * 2 + b]
                n = s1 - s0
                o = seg_off[(w, b)]
                assert n <= T[w, b] * 128
                if n:
                    buck_flag[o:o + n] = bool(b)
                    g_idx[o:o + n] = src_s[s0:s1] - (cfg.lo_total if b else 0)
                    e_a[:3, o:o + n] = ea_s[s0:s1].T
                    e_a[3, o:o + n] = 1.0
                    invd[o:o + n] = inv_s[s0:s1]
                    selcol[o:o + n] = pos_of[dst_s[s0:s1]] - w * WIN

        gi = np.zeros((128, e_pad // 16), np.int16)
        base = g_idx.astype(np.int16).reshape(-1, 16).T
        for k in range(8):
            gi[16 * k:16 * k + 16] = base

        # layer-0 edge stream: [x[src](10) | ea(3) | 1] fp16, feature-major
        xe = np.zeros((node_dim + 4, e_pad), np.float16)
        edge_valid = selcol >= 0
        # recover per-edge src x via the slot->x map
        xe[:node_dim, :] = 0.0

        sel = np.zeros((128, cfg.n_tiles * 128), np.uint8)
        tt = np.arange(e_pad) // 128
        ee = np.arange(e_pad) % 128
        m = selcol >= 0
        sel[ee[m], tt[m] * 128 + selcol[m]] = 0x38

        ppos = pos_of[c * npc:(c + 1) * npc]
        xt = np.zeros((node_dim, win_pad), np.float16)
        xt[:, ppos] = x[c * npc:(c + 1) * npc].astype(np.float16).T
        xt8 = xt.astype(NP_F8)

        sb = np.zeros((128, cfg.n_win * N_GRAPHS), np.uint8)
        bl = batch[c * npc:(c + 1) * npc].astype(np.int64)
        pp = ppos % WIN
        ww = ppos // WIN
        sb[pp, ww * N_GRAPHS + bl] = 0x38

        xsrc_slot = np.zeros((cfg.slots, node_dim), np.float16)
        xsrc_slot[slot_of_node] = x.astype(np.float16)
        gsl = g_idx + np.where(buck_flag, cfg.lo_total, 0)
        xe[:node_dim, :] = xsrc_slot[gsl].T
        xe[node_dim:node_dim + 4, :] = e_a.astype(np.float16)
        xe[:, ~edge_valid] = 0.0

        invw = np.ones((128, cfg.n_win), np.float32)
        iw = inv_deg[c * npc:(c + 1) * npc]
        invw[ppos % WIN, ppos // WIN] = iw

        m_ = {
            "xeT": xe,
            "gidx": gi,
            "eaT": e_a.astype(np.float16),
            "invw": invw,
            "sel": sel.view(NP_F8),
            "xT_loc": xt8,
            "selB": sb.view(NP_F8),
            "ident": ident,
        }
        m_.update(wts)
        in_maps.append(m_)
    return cfg, in_maps


def _pack_weights(inputs, node_dim):
    wts = {}
    node_in = node_dim
    for li in range(len(HID)):
        mw = np.asarray(inputs[f"mw{li}"], np.float32)
        mb = np.asarray(inputs[f"mb{li}"], np.float32)
        wts[f"mwp{li}"] = np.concatenate(
            [mw, mb[None, :]], axis=0).astype(np.float16)
        wts[f"uw{li}"] = np.asarray(inputs[f"uw{li}"], np.float16)
        wts[f"ub{li}"] = np.asarray(inputs[f"ub{li}"], np.float16)[None, :]
        node_in = HID[li]
    for li in range(len(MLP_DIMS) - 1):
        wts[f"fw{li}"] = np.asarray(inputs[f"fw{li}"], np.float16)
        wts[f"fb{li}"] = np.asarray(
            inputs[f"fb{li}"], np.float32).reshape(-1, 1)
    return wts


# =============================================================== bass builder
def build_program(cfg):
    nc = bacc.Bacc(
        "TRN2",
        target_bir_lowering=False,
        debug=False,
        enable_asserts=False,
        num_devices=cfg.n_cores,
        num_swdge_queues=2,
    )
    n_win, npc, n_tiles, e_pad = cfg.n_win, cfg.npc, cfg.n_tiles, cfg.e_pad
    slots, stripes, lo_stripes = cfg.slots, cfg.stripes, cfg.lo_stripes
    win_pad = n_win * WIN
    NG = cfg.n_graphs
    T = cfg.T
    groups = cfg.groups
    seg_off = cfg.seg_off
    core_ids = list(range(cfg.n_cores))

    D = {}

    def din(name, shape, dt):
        D[name] = nc.dram_tensor(name, list(shape), dt, kind="ExternalInput")

    din("xeT", (cfg.node_dim + 4, e_pad), F16)
    din("gidx", (128, e_pad // 16), I16)
    din("eaT", (4, e_pad), F16)
    din("invw", (128, n_win), F32)
    din("sel", (128, n_tiles * 128), F8)
    din("xT_loc", (cfg.node_dim, win_pad), F8)
    din("selB", (128, n_win * NG), F8)
    din("ident", (128, 128), F16)
    node_in = cfg.node_dim
    for li, dout in enumerate(HID):
        din(f"mwp{li}", (node_in + 4, dout), F16)
        din(f"uw{li}", (dout + node_in, dout), F16)
        din(f"ub{li}", (1, dout), F16)
        node_in = dout
    for li in range(len(MLP_DIMS) - 1):
        din(f"fw{li}", (MLP_DIMS[li], MLP_DIMS[li + 1]), F16)
        din(f"fb{li}", (MLP_DIMS[li + 1], 1), F32)
    out_t = nc.dram_tensor("out", [NG, 1], F32, kind="ExternalOutput")

    # group extents in the padded stream
    g_meta = []
    for ws in groups:
        nA = int(sum(T[w, 0] for w in ws)) * 128
        nB = int(sum(T[w, 1] for w in ws)) * 128
        g_meta.append((seg_off[(ws[0], 0)], nA, nB))
    max_g_cols = max(nA + nB for _, nA, nB in g_meta)
    max_w_cols = int((T[:, 0] + T[:, 1]).max()) * 128

    from contextlib import ExitStack
    with ExitStack() as _es:
        tc = _es.enter_context(tile.TileContext(nc))
        p_table = _es.enter_context(tc.tile_pool(name="table", bufs=1))
        p_res = _es.enter_context(tc.tile_pool(name="res", bufs=1))
        p_wts = _es.enter_context(tc.tile_pool(name="wts", bufs=1))
        p_gath = _es.enter_context(tc.tile_pool(name="gath", bufs=3))
        p_sel = _es.enter_context(tc.tile_pool(name="selp", bufs=2))
        p_gix = _es.enter_context(tc.tile_pool(name="gix", bufs=3))
        p_ea = _es.enter_context(tc.tile_pool(name="eal2", bufs=2))
        p_msg = _es.enter_context(tc.tile_pool(name="msg", bufs=2))
        p_aggs = _es.enter_context(tc.tile_pool(name="aggs", bufs=2))
        p_hloc = _es.enter_context(tc.tile_pool(name="hloc", bufs=2))
        p_hn = _es.enter_context(tc.tile_pool(name="hnext", bufs=2))
        pp_msg = _es.enter_context(tc.tile_pool(name="pmsg", bufs=3, space="PSUM"))
        pp_agg = _es.enter_context(tc.tile_pool(name="pagg", bufs=2, space="PSUM"))
        pp_upd = _es.enter_context(tc.tile_pool(name="pupd", bufs=2, space="PSUM"))
        pp_pool = _es.enter_context(tc.tile_pool(name="ppool", bufs=1, space="PSUM"))
        p_dram = _es.enter_context(tc.tile_pool(name="dram", bufs=1, space="DRAM"))
        if True:
            lo_s = lo_stripes
            hi_s = stripes - lo_stripes
            tableA = p_table.tile([128, lo_s * ELEM], F16, tag="tabA")
            tableB = p_table.tile([128, hi_s * ELEM], F16, tag="tabB")
            tblA3 = tableA[:].rearrange("p (s c) -> p s c", c=ELEM)
            tblB3 = tableB[:].rearrange("p (s c) -> p s c", c=ELEM)
            selB_sb = p_res.tile([128, n_win * NG], F8, tag="selB")
            ident_sb = p_res.tile([128, 128], F16, tag="ident")
            ones_row = p_res.tile([1, 128], F16, tag="ones_r")
            ones_col = p_res.tile([128, 1], F16, tag="ones_c")

            nc.sync.dma_start(selB_sb[:], D["selB"][:])
            nc.sync.dma_start(ident_sb[:], D["ident"][:])
            nc.vector.memset(ones_row[:], 1.0)
            nc.vector.memset(ones_col[:], 1.0)

            zeros8 = p_res.tile([128, 128], F8, tag="zeros")
            nc.vector.memset(zeros8[:], 0.0)
            ones32 = p_res.tile([128, 1], F32, tag="ones32")
            nc.vector.memset(ones32[:], 1.0)
            invw_sb = p_res.tile([128, n_win], F32, tag="invw")
            nc.sync.dma_start(invw_sb[:], D["invw"][:])

            # group input staging. All loads ride the Activation HWDGE queue
            # (its SEQ never head-of-line blocks on compute results); the
            # in-order SP queue carries only result writebacks (agi), whose
            # waits then can't delay later groups' input loads. Gather index
            # tiles are loaded two groups ahead so the gather for group g+1
            # can issue while group g computes.
            staged_idx = {}
            staged = {}

            def stage_idx(li, gi_):
                if li == 0 or gi_ >= len(groups) or (li, gi_) in staged_idx:
                    return
                start, nA, nB = g_meta[gi_]
                ncols = nA + nB
                gixb = p_gix.tile([128, max_g_cols // 16], I16, tag="gix")
                nc.scalar.dma_start(
                    gixb[:, 0:ncols // 16],
                    D["gidx"][:, start // 16:(start + ncols) // 16])
                staged_idx[(li, gi_)] = gixb

            def stage_group(li, gi_, node_in):
                if gi_ >= len(groups) or (li, gi_) in staged:
                    return
                start, nA, nB = g_meta[gi_]
                ncols = nA + nB
                selbuf = p_sel.tile([128, max_g_cols], F8, tag="selp")
                nc.scalar.dma_start(
                    selbuf[:, 0:ncols],
                    D["sel"][:, start:start + ncols])
                if li == 0:
                    gbuf = p_gath.tile([cfg.node_dim + 4, max_g_cols],
                                       F16, tag="gath", name="gbuf")
                    nc.scalar.dma_start(
                        gbuf[:, 0:ncols],
                        D["xeT"][:, start:start + ncols])
                else:
                    gbuf = p_gath.tile([128, max_g_cols], F16, tag="gath",
                                       name="gbuf")
                    gixb = staged_idx.pop((li, gi_))
                    for b, coff, n_b in ((0, 0, nA), (1, nA, nB)):
                        if n_b == 0:
                            continue
                        tb = tableA[:] if b == 0 else tableB[:]
                        nc.gpsimd.dma_gather(
                            gbuf[:, coff:coff + n_b].rearrange(
                                "p (o n) -> p o n", o=1),
                            tb,
                            gixb[:, coff // 16:(coff + n_b) // 16],
                            queue_num=b,
                            num_idxs=n_b,
                            num_idxs_reg=n_b,
                            elem_size=ELEM,
                            transpose=True,
                            single_packet=False,
                            sbuf_tokens_per_rank=128,
                            sbuf_free_dim_per_rank=ELEM * 2,
                        )
                    if node_in + 4 <= 128:
                        nc.scalar.dma_start(
                            gbuf[node_in:node_in + 4, 0:ncols],
                            D["eaT"][:, start:start + ncols])
                staged[(li, gi_)] = (selbuf, gbuf)

            # group-0 inputs staged before the weight DMAs so the first
            # matmul's operands win the HWDGE queue
            stage_group(0, 0, cfg.node_dim)

            # weights -> SBUF (pre-cast to fp16 on host; HWDGE loads keep the
            # Pool queue free for gathers/collectives). Loaded lazily one
            # layer ahead.
            W = {}

            def load_wts(li):
                dout = HID[li]
                node_in = cfg.node_dim if li == 0 else HID[li - 1]
                mw_chunks = []
                for k, r in enumerate(range(0, node_in + 4, 128)):
                    r1 = min(r + 128, node_in + 4)
                    t = p_wts.tile([r1 - r, dout], F16, tag=f"mwp{li}_{k}")
                    nc.scalar.dma_start(t[:], D[f"mwp{li}"][r:r1, :])
                    mw_chunks.append(t)
                W[f"mwp{li}"] = mw_chunks
                # uw chunks: agg rows [0:dout] in 128-chunks, then h rows
                chunks = []
                for r in list(range(0, dout, 128)):
                    chunks.append((r, min(r + 128, dout)))
                for r in list(range(0, node_in, 128)):
                    chunks.append((dout + r, dout + min(r + 128, node_in)))
                uws = []
                for k, (r0, r1) in enumerate(chunks):
                    t = p_wts.tile([r1 - r0, dout], F16, tag=f"uw{li}_{k}")
                    nc.scalar.dma_start(t[:], D[f"uw{li}"][r0:r1, :])
                    uws.append(t)
                W[f"uw{li}"] = uws
                t = p_wts.tile([1, dout], F16, tag=f"ub{li}")
                nc.scalar.dma_start(t[:], D[f"ub{li}"][:])
                W[f"ub{li}"] = t

            def load_fw():
                for li in range(len(MLP_DIMS) - 1):
                    fws = []
                    for k, r in enumerate(range(0, MLP_DIMS[li], 128)):
                        r1 = min(r + 128, MLP_DIMS[li])
                        t = p_wts.tile([r1 - r, MLP_DIMS[li + 1]], F16,
                                       tag=f"fw{li}_{k}")
                        nc.scalar.dma_start(t[:], D[f"fw{li}"][r:r1, :])
                        fws.append(t)
                    W[f"fw{li}"] = fws
                    t = p_wts.tile([MLP_DIMS[li + 1], 1], F32, tag=f"fb{li}")
                    nc.sync.dma_start(t[:], D[f"fb{li}"][:])
                    W[f"fb{li}"] = t

            load_wts(0)
            hlocT = p_hloc.tile([cfg.node_dim, win_pad], F8, tag="hloc")
            nc.sync.dma_start(hlocT[:], D["xT_loc"][:])
            # table init: only cols HID[0]:ELEM are ever read before being
            # written (L1 gathers pull full 256B slots; the rebuild covers
            # cols 0:dout). gpsimd halves keep DVE free for layer-0 compute;
            # Pool is idle until the first collective (~30us).
            nc.gpsimd.memset(tblA3[:, :, HID[0]:ELEM], 0.0)
            nc.gpsimd.memset(tblB3[:, :, HID[0]:ELEM], 0.0)

            lo_w = cfg.lo // 128
            assert cfg.hi_total > 0
            hi_nw = n_win - lo_w
            amid = max(1, (lo_w * 2) // 5)
            h1 = min(n_win, lo_w + max(1, hi_nw // 2))
            chunk_bounds = [(0, amid), (amid, lo_w),
                            (lo_w, h1), (h1, n_win)]
            chunk_bounds = [(a, b) for a, b in chunk_bounds if b > a]
            chunk_of_w = {}
            for ci, (a, b) in enumerate(chunk_bounds):
                for w in range(a, b):
                    chunk_of_w[w] = ci
            ag_in = {}
            ag_out = {}
            for li in (0, 1):
                for ci, (a, b) in enumerate(chunk_bounds):
                    ag_in[(li, ci)] = p_dram.tile(
                        [128, (b - a) * HID[li]], F8,
                        tag=f"agi{li}_{ci}", name=f"agi{li}_{ci}")
                    ag_out[(li, ci)] = p_dram.tile(
                        [cfg.n_cores * 128, (b - a) * HID[li]], F8,
                        tag=f"ago{li}_{ci}", name=f"ago{li}_{ci}",
                        addr_space="Shared")
            FP = HID[2] + 1
            gp_in = p_dram.tile([NG, FP], F16, tag="gpi")
            if DEBUG:
                dbg_h3 = p_dram.tile([128, n_win * HID[2]], F16, tag="dbgh3",
                                     name="dbg_h3")
                dbg_ag = p_dram.tile([128, n_win * HID[2]], F16, tag="dbgag",
                                     name="dbg_ag")
                dbg_msg = p_dram.tile([128, 16 * HID[2]], F16, tag="dbgmsg",
                                      name="dbg_msg")
            gp_out = p_dram.tile([cfg.n_cores * NG, FP], F16, tag="gpo", addr_space="Shared")

            psum_pool = pp_pool.tile([NG, FP], F32)

            lo_s_pc = cfg.lo // 128
            hi_s_pc = n_win - lo_s_pc

            def _ag_emit(li, ci, dout):
                nc.gpsimd.collective_compute(
                    "AllGather",
                    mybir.AluOpType.bypass,
                    replica_groups=[core_ids],
                    ins=[ag_in[(li, ci)].opt()],
                    outs=[ag_out[(li, ci)].opt()],
                )

            tblA4 = tableA[:].rearrange(
                "p (r s c) -> p r s c", r=cfg.n_cores, c=ELEM)
            tblB4 = tableB[:].rearrange(
                "p (r s c) -> p r s c", r=cfg.n_cores, c=ELEM)

            def _rebuild_chunk(li, ci, dout):
                # gpsimd (SWDGE, casting fp8->fp16) DMAs: per-core fragment
                # -> that core's stripe range of the table. When dout == ELEM
                # the target range is fully contiguous (few descriptors).
                a, b = chunk_bounds[ci]
                nw = b - a
                for rr in range(cfg.n_cores):
                    frag = ag_out[(li, ci)][rr * 128:(rr + 1) * 128, :]
                    if b <= lo_w:
                        s0 = rr * lo_s_pc + a
                        if dout == ELEM:
                            nc.gpsimd.dma_start(
                                tableA[:, s0 * ELEM:(s0 + nw) * ELEM],
                                frag[:])
                        else:
                            nc.gpsimd.dma_start(
                                tblA3[:, s0:s0 + nw, 0:dout],
                                frag[:].rearrange("p (s c) -> p s c", c=dout))
                    else:
                        s0 = rr * hi_s_pc + (a - lo_w)
                        if dout == ELEM:
                            nc.gpsimd.dma_start(
                                tableB[:, s0 * ELEM:(s0 + nw) * ELEM],
                                frag[:])
                        else:
                            nc.gpsimd.dma_start(
                                tblB3[:, s0:s0 + nw, 0:dout],
                                frag[:].rearrange("p (s c) -> p s c", c=dout))

            # ========================= layers
            node_in = cfg.node_dim
            for li, dout in enumerate(HID):
                mwp = W[f"mwp{li}"]
                nch = _cdiv(dout, 128)
                last = li == len(HID) - 1
                if not last:
                    load_wts(li + 1)
                else:
                    load_fw()
                hnextT = None
                if not last:
                    hnextT = p_hloc.tile([dout, win_pad], F8, tag="hloc")

                rebuilt = set()
                stage_idx(li, 0)
                stage_idx(li, 1)
                stage_group(li, 0, node_in)
                stage_group(li, 1, node_in)
                for gi_, ws in enumerate(groups):
                    start, nA, nB = g_meta[gi_]
                    ncols = nA + nB
                    selbuf, gbuf = staged.pop((li, gi_))
                    stage_idx(li, gi_ + 2)
                    stage_group(li, gi_ + 1, node_in)
                    if li == 1 and gi_ == len(groups) - 1:
                        # all of this layer's gathers are emitted; chunks
                        # whose exchange is also already emitted can rebuild
                        # now, overlapping the tail windows
                        for ci, (a, b) in enumerate(chunk_bounds):
                            if b - 1 < ws[0]:
                                _rebuild_chunk(li, ci, dout)
                                rebuilt.add(ci)

                    # window-major processing; stream offsets are bucket-major
                    aoff = 0
                    boff = nA
                    for w in ws:
                        t0, t1 = int(T[w, 0]), int(T[w, 1])
                        wcols = []
                        for k in range(t0):
                            wcols.append(aoff + k * 128)
                        for k in range(t1):
                            wcols.append(boff + k * 128)
                        aoff += t0 * 128
                        boff += t1 * 128

                        eabuf = None
                        if node_in + 4 > 128:
                            eabuf = p_ea.tile([4, max_w_cols], F16, tag="eal2")
                            nc.sync.dma_start(
                                eabuf[:, 0:t0 * 128],
                                D["eaT"][:, start + wcols[0]:
                                         start + wcols[0] + t0 * 128])
                            nc.sync.dma_start(
                                eabuf[:, t0 * 128:(t0 + t1) * 128],
                                D["eaT"][:, start + wcols[t0]:
                                         start + wcols[t0] + t1 * 128])

                        # node-major aggregation: sel is the stationary
                        # operand, messages stream; the mean's 1/deg scale
                        # is applied once per window (per-partition scalar)
                        # so message activations batch up to 512 cols
                        pagg = pp_agg.tile([128, max(HID)], F32, tag="pagg")
                        n_wt = len(wcols)
                        BATCH = 512 // dout
                        pend = None

                        def _agg_emit(bcols, msgb):
                            for j, col in enumerate(bcols):
                                nc.tensor.matmul(
                                    pagg[:, 0:dout],
                                    selbuf[:, col:col + 128],
                                    msgb[:, j * dout:(j + 1) * dout],
                                    start=(col == wcols[0]),
                                    stop=(col == wcols[-1]),
                                    skip_group_check=True)

                        for b0 in range(0, n_wt, BATCH):
                            bcols = wcols[b0:b0 + BATCH]
                            nbd = len(bcols) * dout
                            pmsg = pp_msg.tile([128, 512], F32, tag="pmsg")
                            for j, col in enumerate(bcols):
                                o = j * dout
                                if node_in + 4 <= 128:
                                    nc.tensor.matmul(
                                        pmsg[:, o:o + dout],
                                        gbuf[0:node_in + 4, col:col + 128],
                                        mwp[0][:], start=True, stop=True,
                                        skip_group_check=True)
                                else:
                                    k = b0 + j
                                    nc.tensor.matmul(
                                        pmsg[:, o:o + dout],
                                        gbuf[0:128, col:col + 128],
                                        mwp[0][:], start=True, stop=False,
                                        skip_group_check=True)
                                    nc.tensor.matmul(
                                        pmsg[:, o:o + dout],
                                        eabuf[:, k * 128:k * 128 + 128],
                                        mwp[1][:], start=False, stop=True,
                                        skip_group_check=True)
                            msgb = p_msg.tile([128, 512], F16, tag="msg")
                            if (b0 // BATCH) % 2 == 0:
                                nc.scalar.activation(
                                    msgb[:, 0:nbd], pmsg[:, 0:nbd], AF.Prelu,
                                    alpha=LRELU)
                            else:
                                nc.vector._custom_dve(
                                    _get_lrelu_op(), out=msgb[:, 0:nbd],
                                    in0=pmsg[:, 0:nbd], s0=ones32[:],
                                    imm2=LRELU)
                            if pend is not None:
                                _agg_emit(*pend)
                            pend = (bcols, msgb)
                        if pend is not None:
                            _agg_emit(*pend)

                        # ----- window update
                        wn = WIN if w < n_win - 1 else cfg.last_win_nodes
                        agg_sb = p_aggs.tile([128, max(HID)], F16, tag="agsb")
                        if w % 2 == 0:
                            nc.scalar.activation(
                                agg_sb[:, 0:dout], pagg[:, 0:dout],
                                AF.Identity, scale=invw_sb[:, w:w + 1])
                        else:
                            nc.vector._custom_dve(
                                _get_lrelu_op(), out=agg_sb[:, 0:dout],
                                in0=pagg[:, 0:dout],
                                s0=invw_sb[:, w:w + 1], imm2=1.0)
                        lhs_list = []
                        for ch in range(nch):
                            c0, c1 = ch * 128, min(dout, ch * 128 + 128)
                            # transpose as a plain matmul against identity:
                            # a transpose-mode matmul would be serialized
                            # against in-flight collectives by the scheduler
                            pt = pp_upd.tile([128, 128], F32, tag="pupd")
                            nc.tensor.matmul(
                                pt[0:c1 - c0, :], agg_sb[:, c0:c1],
                                ident_sb[:], start=True, stop=True,
                                skip_group_check=True)
                            a = p_aggs.tile([128, 128], F16, tag="aggs")
                            nc.vector.tensor_copy(
                                a[0:c1 - c0, :], pt[0:c1 - c0, :])
                            lhs_list.append(a[0:c1 - c0, :])
                        for r in range(0, node_in, 128):
                            r1 = min(node_in, r + 128)
                            lhs_list.append(
                                hlocT[r:r1, w * WIN:w * WIN + 128])
                        uws = W[f"uw{li}"]
                        assert len(uws) == len(lhs_list)
                        pupd = pp_upd.tile([128, max(HID)], F32, tag="pupd")
                        for i, lhs in enumerate(lhs_list):
                            nc.tensor.matmul(
                                pupd[:, 0:dout], lhs, uws[i][:],
                                start=(i == 0), stop=False)
                        nc.tensor.matmul(
                            pupd[:, 0:dout], ones_row[:], W[f"ub{li}"][:],
                            start=False, stop=True)
                        hn = p_hn.tile([128, max(HID)], F16, tag="hnext")
                        nc.scalar.activation(
                            hn[:, 0:dout], pupd[:, 0:dout], AF.Prelu,
                            alpha=LRELU)
                        if not last:
                            hn8 = p_hn.tile([128, 128], F8, tag="hn8")
                            nc.vector.tensor_copy(
                                hn8[:, 0:dout], hn[:, 0:dout])
                            ci = chunk_of_w[w]
                            wl = w - chunk_bounds[ci][0]
                            agi = ag_in[(li, ci)]
                            eng_agi = nc.sync
                            eng_agi.dma_start(
                                agi[:, wl * dout:wl * dout + dout]
                                if wn == WIN else
                                agi[0:wn, wl * dout:wl * dout + dout],
                                hn8[:, 0:dout] if wn == WIN
                                else hn8[0:wn, 0:dout])
                            if wn < WIN:
                                eng_agi.dma_start(
                                    agi[wn:128,
                                        wl * dout:wl * dout + dout],
                                    zeros8[wn:128, 0:dout])
                            for ch in range(nch):
                                c0, c1 = ch * 128, min(dout, ch * 128 + 128)
                                pt = pp_upd.tile([128, 128], F32, tag="pupd")
                                nc.tensor.matmul(
                                    pt[0:c1 - c0, :], hn[:, c0:c1],
                                    ident_sb[:], start=True, stop=True,
                                    skip_group_check=True)
                                nc.vector.tensor_copy(
                                    hnextT[c0:c1, w * WIN:w * WIN + 128],
                                    pt[0:c1 - c0, :])
                            if w == chunk_bounds[ci][1] - 1:
                                _ag_emit(li, ci, dout)
                                if li == 0 or gi_ == len(groups) - 1:
                                    # L0 has no gathers; in the last group
                                    # every gather is already emitted -> the
                                    # rebuild can follow the exchange at once
                                    _rebuild_chunk(li, ci, dout)
                                    rebuilt.add(ci)
                        else:
                            if DEBUG:
                                nc.sync.dma_start(
                                    dbg_h3[:, w * dout:(w + 1) * dout],
                                    hn[:, 0:dout])
                                for _ch in range(nch):
                                    _cw = (min(dout, _ch * 128 + 128)
                                           - _ch * 128)
                                    nc.sync.dma_start(
                                        dbg_ag[0:_cw,
                                               (w * 2 + _ch) * 128:
                                               (w * 2 + _ch) * 128 + 128],
                                        lhs_list[_ch])
                            nc.tensor.matmul(
                                psum_pool[:, 0:dout],
                                selB_sb[:, w * NG:(w + 1) * NG],
                                hn[:, 0:dout],
                                start=(w == 0), stop=False,
                                skip_group_check=True)
                            nc.tensor.matmul(
                                psum_pool[:, dout:dout + 1],
                                selB_sb[:, w * NG:(w + 1) * NG],
                                ones_col[:],
                                start=False, stop=(w == n_win - 1),
                                skip_group_check=True)

                # ----- end of layer
                if not last:
                    for ci in range(len(chunk_bounds)):
                        if ci not in rebuilt:
                            _rebuild_chunk(li, ci, dout)
                    hlocT = hnextT
                node_in = dout

            # ========================= pooling finale + MLP (replicated)
            gp_sb = p_msg.tile([NG, FP], F16, tag="msg", name="gp_sb")
            nc.vector.tensor_copy(gp_sb[:], psum_pool[:])
            nc.sync.dma_start(gp_in[:], gp_sb[:])
            nc.gpsimd.collective_compute(
                "AllGather",
                mybir.AluOpType.bypass,
                replica_groups=[core_ids],
                ins=[gp_in.opt()],
                outs=[gp_out.opt()],
            )
            # cross-core reduce on TensorE: two 4-core chunks of the gathered
            # partials, contracted against a stacked 32-identity
            gpc = p_sel.tile([128, 2 * FP], F16, tag="selp", name="gpc")
            for k in range(2):
                nc.sync.dma_start(
                    gpc[:, k * FP:(k + 1) * FP],
                    gp_out[k * 128:(k + 1) * 128, :])
            i32 = p_aggs.tile([128, NG], F16, tag="aggs")
            for k in range(4):
                src_blk = ident_sb[:, k * NG:(k + 1) * NG]
                if k == 0:
                    nc.vector.tensor_copy(i32[:], src_blk)
                else:
                    nc.vector.tensor_tensor(
                        i32[:], i32[:], src_blk, op=mybir.AluOpType.add)
            psum_gs = pp_upd.tile([NG, FP], F32, tag="pupd")
            for k in range(2):
                nc.tensor.matmul(
                    psum_gs[:, 0:FP], i32[:, 0:NG],
                    gpc[:, k * FP:(k + 1) * FP],
                    start=(k == 0), stop=(k == 1))
            rec = p_aggs.tile([NG, 1], F32, tag="aggs")
            nc.vector.reciprocal(rec[:], psum_gs[:, HID[2]:HID[2] + 1])
            g16 = p_msg.tile([NG, HID[2]], F16, tag="msg")
            nc.scalar.activation(g16[:], psum_gs[:, 0:HID[2]], AF.Identity,
                                 scale=rec[:])
            gT = p_msg.tile([128, 2 * NG], F16, tag="msg")
            for ch in range(2):
                pt = pp_upd.tile([128, 128], F32, tag="pupd")
                nc.tensor.matmul(
                    pt[0:128, 0:NG], g16[:, ch * 128:(ch + 1) * 128],
                    ident_sb[0:NG, 0:NG], start=True, stop=True,
                    skip_group_check=True)
                nc.vector.tensor_copy(
                    gT[:, ch * NG:(ch + 1) * NG], pt[0:128, 0:NG])
            pf1 = pp_upd.tile([128, NG], F32, tag="pupd")
            for ch in range(2):
                nc.tensor.matmul(
                    pf1[0:MLP_DIMS[1], :], W["fw0"][ch][:],
                    gT[:, ch * NG:(ch + 1) * NG],
                    start=(ch == 0), stop=(ch == 1))
            f1 = p_aggs.tile([MLP_DIMS[1], NG], F16, tag="aggs")
            nc.scalar.activation(f1[:], pf1[0:MLP_DIMS[1], :], AF.Prelu,
                                 bias=W["fb0"][:], alpha=LRELU)
            pf2 = pp_upd.tile([128, NG], F32, tag="pupd")
            nc.tensor.matmul(pf2[0:MLP_DIMS[2], :], W["fw1"][0][:], f1[:],
                             start=True, stop=True)
            f2 = p_aggs.tile([MLP_DIMS[2], NG], F16, tag="aggs")
            nc.scalar.activation(f2[:], pf2[0:MLP_DIMS[2], :], AF.Prelu,
                                 bias=W["fb1"][:], alpha=LRELU)
            pf3 = pp_upd.tile([128, NG], F32, tag="pupd")
            nc.tensor.matmul(pf3[0:1, :], W["fw2"][0][:], f2[:],
                             start=True, stop=True)
            fout = p_aggs.tile([1, NG], F32, tag="aggs")
            nc.scalar.activation(fout[:], pf3[0:1, :], AF.Identity,
                                 bias=W["fb2"][:])
            nc.sync.dma_start(out_t[:].rearrange("g o -> o g"), fout[:])

    return nc


def kernel(**inputs):
    cfg, in_maps = host_prep(inputs)
    nc = build_program(cfg)
    nc.compile()
    res = run_bass_kernel_spmd(nc, in_maps, core_ids=list(range(cfg.n_cores)))
    return np.asarray(res.results[0]["out"], np.float32)

